# revision 11
# baseline (speedup 1.0000x reference)
"""GraphSAGE (2-layer, mean aggregation) on 8 Trainium2 NeuronCores.

Strategy: destination nodes are sharded across the 8 cores (49 tiles of 128
nodes per core, LPT-balanced by degree). Edges are partitioned by destination
tile, padded to a uniform chunk count T per tile so one SPMD program serves
all cores.

Per-edge source rows are fetched with SWDGE `dma_gather` (two 1024-index
instructions per tile — 1024 descriptors is the per-instruction SWDGE scratch
cap; larger gathers fault the device). dma_gather uses int16 indices (max
32767), so each core's 49 tiles are split into index groups whose unique
source-node sets fit in a 32768-row gather table; tables hold bf16 feature
rows (256 B, the SWDGE minimum element). Measured Q7 descriptor generation
runs ~9 ns/row serially per core, which makes the gathers the span-defining
cost of both layers; the PE/DVE/Scalar work and the DMA transfers themselves
all hide underneath.

The segment sum for a destination tile runs on the PE in bf16: a 0/1
selection matrix S[e, n] = (dst_slot[e] == n) is formed on the vector engine
(iota compare) and S^T @ messages accumulates into PSUM over the tile's
chunks. Mean division, dense lin_l/lin_r matmuls, bias and ReLU happen
on-device in bf16 (fp32 PSUM accumulate).

Layer 2 exploits linearity: p = h @ W2l^T ([N, 64]) is computed at the end of
layer 1 (per-core, own nodes), so layer-2 messages are 64-wide instead of
256-wide — 4x less gather traffic and PE work. Layer 2 then only needs
mean-aggregate(p) + h @ W2r^T + b2, with the bias added via a K=1 matmul
(ones ⊗ b2) into the same PSUM accumulation.

The host does integer index preprocessing, sharding/layout, bf16 casts and
un-sharding; all float tensor math runs on the NeuronCores.
"""
import heapq
import sys
from contextlib import ExitStack

import numpy as np
import ml_dtypes

for _p in ("/opt/trn_rl_repo",):
    if _p not in sys.path:
        sys.path.insert(0, _p)

import concourse.tile as tile
from concourse import bacc, mybir
from concourse.bass_utils import run_bass_kernel_spmd

BF16 = ml_dtypes.bfloat16


def _ensure_axon_hooks():
    """run_bass_kernel_spmd(trace=True) imports antenv.axon_hooks, which this
    image lacks; install a ctypes-backed hook so tracing works (or degrades
    to a no-op instead of an ImportError)."""
    try:
        import antenv.axon_hooks  # noqa: F401
        return
    except ImportError:
        pass
    import contextlib
    import ctypes
    import types

    def _make_hook():
        try:
            lib = ctypes.CDLL("/opt/axon/libaxon_pjrt.so")
        except OSError:
            return None
        if not hasattr(lib, "axon_start_nrt_profile"):
            return None
        lib.axon_start_nrt_profile.argtypes = [ctypes.POINTER(ctypes.c_int64), ctypes.c_size_t]
        lib.axon_start_nrt_profile.restype = ctypes.c_int64
        lib.axon_stop_nrt_profile.argtypes = [ctypes.c_char_p]
        lib.axon_stop_nrt_profile.restype = ctypes.c_int64

        @contextlib.contextmanager
        def _hook(output_dir, device_ids):
            import jax
            jax.devices()
            if device_ids:
                ids = (ctypes.c_int64 * len(device_ids))(*device_ids)
                rc = lib.axon_start_nrt_profile(ids, len(device_ids))
            else:
                rc = lib.axon_start_nrt_profile(None, 0)
            if rc != 0:
                raise RuntimeError(f"axon_start_nrt_profile rc={rc}")
            try:
                yield
            finally:
                lib.axon_stop_nrt_profile(str(output_dir).encode())

        return _hook

    hook = _make_hook()
    mod = types.ModuleType("antenv.axon_hooks")
    mod.get_axon_ntff_profile_hook = lambda: hook
    mod.set_axon_ntff_profile_hook = lambda h: None
    import antenv
    antenv.axon_hooks = mod
    sys.modules["antenv.axon_hooks"] = mod


_ensure_axon_hooks()


def _run_spmd_retry(nc, in_maps, **kw):
    """Retries for transient NRT device errors (axon cores report
    EXEC_UNIT_UNRECOVERABLE for ~60-120 s after a prior faulted run)."""
    import time
    for wait in (75, 120):
        try:
            return run_bass_kernel_spmd(nc, in_maps, core_ids=list(range(N_CORES)), **kw)
        except Exception:
            time.sleep(wait)
    return run_bass_kernel_spmd(nc, in_maps, core_ids=list(range(N_CORES)), **kw)

N_NODES = 50000
N_EDGES = 800000
DIM_IN, DIM_H, DIM_OUT = 128, 256, 64
N_CORES = 8
P = 128
TILES_PER_CORE = 49                      # ceil(50000 / 8 / 128)
N_TILES = N_CORES * TILES_PER_CORE       # 392
NPAD_CORE = TILES_PER_CORE * P           # 6272
PAD_SLOT = 200.0                         # dst_rel sentinel: matches no iota lane
TBL_ROWS = 32768                         # int16 gather-table row limit
GCH = 8                                  # chunks per dma_gather (<=1024 descs/inst)

LAST_RESULTS = []   # test harness reads profiling results from here


def _partition_nodes(deg):
    """LPT-pack nodes into N_TILES bins of <=128 nodes, minimizing max bin
    degree-sum. Returns (tile_of, slot_of, T) with T = uniform chunks/tile."""
    order = np.argsort(-deg, kind="stable")
    heap = [(0, t) for t in range(N_TILES)]
    heapq.heapify(heap)
    counts = np.zeros(N_TILES, np.int64)
    sums = np.zeros(N_TILES, np.int64)
    tile_of = np.empty(N_NODES, np.int64)
    slot_of = np.empty(N_NODES, np.int64)
    for node in order:
        while True:
            s, t = heapq.heappop(heap)
            if counts[t] < P:
                break
        tile_of[node] = t
        slot_of[node] = counts[t]
        counts[t] += 1
        sums[t] += deg[node]
        if counts[t] < P:
            heapq.heappush(heap, (sums[t], t))
    T = int(np.ceil(sums.max() / P))
    return tile_of, slot_of, T


def _build_edge_layout(src, dst, tile_of, slot_of, T):
    """Per-core chunk-major index arrays.

    Returns src_cols, dst_cols: lists (per core) of [P, 49*T] arrays where
    column t*T + j holds chunk j of tile t: lane p is edge j*128+p of that
    tile's padded edge list (src node id / dst slot, PAD entries src=0,
    dst_rel=PAD_SLOT).
    """
    etile = tile_of[dst]
    order = np.argsort(etile, kind="stable")
    counts = np.bincount(etile, minlength=N_TILES)
    src_pad = np.zeros((N_TILES, T * P), np.int64)
    dst_pad = np.full((N_TILES, T * P), PAD_SLOT, np.float32)
    rank = np.arange(N_EDGES) - np.repeat(np.concatenate([[0], np.cumsum(counts)[:-1]]), counts)
    es, ed = src[order], dst[order]
    src_pad[etile[order], rank] = es
    dst_pad[etile[order], rank] = slot_of[ed]
    src_cols, dst_cols = [], []
    for c in range(N_CORES):
        sl = slice(c * TILES_PER_CORE, (c + 1) * TILES_PER_CORE)
        s = src_pad[sl].reshape(TILES_PER_CORE, T, P).transpose(2, 0, 1).reshape(P, TILES_PER_CORE * T)
        d = dst_pad[sl].reshape(TILES_PER_CORE, T, P).transpose(2, 0, 1).reshape(P, TILES_PER_CORE * T)
        src_cols.append(np.ascontiguousarray(s))
        dst_cols.append(np.ascontiguousarray(d))
    return src_cols, dst_cols


def _pick_groups(src_cols, dst_cols, T):
    """Split each core's 49 tiles into contiguous groups whose unique
    source sets fit a 32768-row table. Greedy over tiles, max over cores so
    group bounds are uniform (SPMD). Returns list of (start_tile, end_tile)."""
    bounds = []
    start = 0
    while start < TILES_PER_CORE:
        end = start + 1
        while end < TILES_PER_CORE:
            ok = True
            for c in range(N_CORES):
                blk = src_cols[c][:, start * T:(end + 1) * T]
                pad = dst_cols[c][:, start * T:(end + 1) * T] == PAD_SLOT
                n_uniq = len(np.unique(blk[~pad]))
                if n_uniq > TBL_ROWS - 8:
                    ok = False
                    break
            if not ok:
                break
            end += 1
        bounds.append((start, end))
        start = end
    return bounds


def _build_gather_meta(src_cols, dst_cols, T, bounds):
    """Per (core, group): unique source node list + int16 index array.

    Returns uniqs[c][g] (node ids) and idx16[c] [P, 49*P] int16 where tile
    t's block [:, t*P:(t+1)*P] holds the dma_gather index layout: edge
    i = j*128+p of the tile maps to [i % 16 (replicated x8), t*P + i//16].
    """
    uniqs = [[None] * len(bounds) for _ in range(N_CORES)]
    idx16 = []
    for c in range(N_CORES):
        out = np.zeros((P, TILES_PER_CORE * P), np.int16)
        for g, (t0, t1) in enumerate(bounds):
            blk = src_cols[c][:, t0 * T:t1 * T]
            pad = dst_cols[c][:, t0 * T:t1 * T] == PAD_SLOT
            uniq = np.unique(blk[~pad]) if (~pad).any() else np.array([0], np.int64)
            assert len(uniq) <= TBL_ROWS, f"group {g} core {c}: {len(uniq)} uniques"
            remap = np.zeros(N_NODES, np.int64)
            remap[uniq] = np.arange(len(uniq))
            uniqs[c][g] = uniq
            for t in range(t0, t1):
                arr = src_cols[c][:, t * T:(t + 1) * T]          # [P, T] lane p chunk j
                padm = dst_cols[c][:, t * T:(t + 1) * T] == PAD_SLOT
                r = remap[arr]
                r[padm] = 0
                flat = r.T.reshape(-1)                            # i = j*128+p order
                blk16 = flat.reshape(T * P // 16, 16).T           # [16, T*P/16]
                out[:, t * P:(t + 1) * P] = np.tile(blk16, (8, 1)).astype(np.int16)
        idx16.append(out)
    return uniqs, idx16


def _build_layer1(T, bounds):
    """Layer 1 + p-pretransform as an SPMD bass program."""
    NG = len(bounds)
    nc = bacc.Bacc("TRN2", target_bir_lowering=False, debug=False,
                   enable_asserts=False, num_devices=N_CORES,
                   dynamic_dma_scratch_size=65536)
    dt = mybir.dt
    tbls = [nc.dram_tensor(f"tbl{g}", [TBL_ROWS, P], dt.bfloat16, kind="ExternalInput").ap()
            for g in range(NG)]
    idx = nc.dram_tensor("idx", [P, TILES_PER_CORE * P], dt.int16, kind="ExternalInput").ap()
    dst_rel = nc.dram_tensor("dst_rel", [P, TILES_PER_CORE * T], dt.bfloat16, kind="ExternalInput").ap()
    deg_col = nc.dram_tensor("deg_col", [P, TILES_PER_CORE], dt.float32, kind="ExternalInput").ap()
    selfT = nc.dram_tensor("selfT", [P, NPAD_CORE], dt.bfloat16, kind="ExternalInput").ap()
    wl = nc.dram_tensor("wl", [P, DIM_H], dt.bfloat16, kind="ExternalInput").ap()
    wr = nc.dram_tensor("wr", [P, DIM_H], dt.bfloat16, kind="ExternalInput").ap()
    b1 = nc.dram_tensor("b1", [P, 2], dt.float32, kind="ExternalInput").ap()
    w2lT = nc.dram_tensor("w2lT", [P, DIM_OUT * 2], dt.bfloat16, kind="ExternalInput").ap()
    iota = nc.dram_tensor("iota", [P, T * P], dt.bfloat16, kind="ExternalInput").ap()
    identity = nc.dram_tensor("identity", [P, P], dt.bfloat16, kind="ExternalInput").ap()
    hT = nc.dram_tensor("hT", [DIM_H, NPAD_CORE], dt.bfloat16, kind="ExternalOutput").ap()
    pT = nc.dram_tensor("pT", [DIM_OUT, NPAD_CORE], dt.bfloat16, kind="ExternalOutput").ap()

    with tile.TileContext(nc) as tc:
        with ExitStack() as ctx:
            const = ctx.enter_context(tc.tile_pool(name="const", bufs=1))
            msgp = ctx.enter_context(tc.tile_pool(name="msgp", bufs=2))
            sp = ctx.enter_context(tc.tile_pool(name="sp", bufs=2))
            work = ctx.enter_context(tc.tile_pool(name="work", bufs=2))
            outp = ctx.enter_context(tc.tile_pool(name="outp", bufs=3))
            psA = ctx.enter_context(tc.tile_pool(name="psA", bufs=2, space="PSUM"))
            psB = ctx.enter_context(tc.tile_pool(name="psB", bufs=2, space="PSUM"))
            psC = ctx.enter_context(tc.tile_pool(name="psC", bufs=2, space="PSUM"))
            psD = ctx.enter_context(tc.tile_pool(name="psD", bufs=2, space="PSUM"))

            warm_idx = const.tile([P, 8], dt.int16)
            nc.vector.memset(warm_idx[:], 0)
            warm_out = const.tile([P, 1, P], dt.bfloat16)
            nc.gpsimd.dma_gather(
                out_ap=warm_out[:], in_ap=tbls[0][:, :], idxs_ap=warm_idx[:],
                num_idxs=P, num_idxs_reg=P, elem_size=P,
            )
            idx_sb = const.tile([P, TILES_PER_CORE * P], dt.int16)
            nc.sync.dma_start(idx_sb[:], idx[:, :])
            dr_sb = const.tile([P, TILES_PER_CORE * T], dt.bfloat16)
            nc.sync.dma_start(dr_sb[:], dst_rel[:, :])
            deg_sb = const.tile([P, TILES_PER_CORE], dt.float32)
            nc.sync.dma_start(deg_sb[:], deg_col[:, :])
            self_sb = const.tile([P, NPAD_CORE], dt.bfloat16)
            nc.sync.dma_start(self_sb[:], selfT[:, :])
            wl_sb = const.tile([P, DIM_H], dt.bfloat16)
            nc.sync.dma_start(wl_sb[:], wl[:, :])
            wr_sb = const.tile([P, DIM_H], dt.bfloat16)
            nc.sync.dma_start(wr_sb[:], wr[:, :])
            b1_sb = const.tile([P, 2], dt.float32)
            nc.sync.dma_start(b1_sb[:], b1[:, :])
            w2l_sb = const.tile([P, DIM_OUT * 2], dt.bfloat16)
            nc.sync.dma_start(w2l_sb[:], w2lT[:, :])
            iota_sb = const.tile([P, T * P], dt.bfloat16)
            nc.sync.dma_start(iota_sb[:], iota[:, :])
            ident = const.tile([P, P], dt.bfloat16)
            nc.sync.dma_start(ident[:], identity[:, :])

            recip = const.tile([P, TILES_PER_CORE], dt.float32)
            nc.vector.tensor_scalar_max(recip[:], deg_sb[:], 1.0)
            nc.vector.reciprocal(recip[:], recip[:])

            for t in range(TILES_PER_CORE):
                g = next(i for i, (t0, t1) in enumerate(bounds) if t0 <= t < t1)
                msgs = msgp.tile([P, T, P], dt.bfloat16)
                for q in range(0, T, GCH):
                    qe = min(q + GCH, T)
                    nc.gpsimd.dma_gather(
                        out_ap=msgs[:, q:qe, :],
                        in_ap=tbls[g][:, :],
                        idxs_ap=idx_sb[:, t * P + q * (P // 16):t * P + qe * (P // 16)],
                        num_idxs=(qe - q) * P,
                        num_idxs_reg=(qe - q) * P,
                        elem_size=P,
                    )
                S = sp.tile([P, T * P], dt.bfloat16)
                try:
                    nc.vector.tensor_tensor(
                        out=S[:],
                        in0=dr_sb[:, t * T:(t + 1) * T, None].to_broadcast([P, T, P]),
                        in1=iota_sb[:],
                        op=mybir.AluOpType.is_equal,
                    )
                except Exception:
                    for j in range(T):
                        nc.vector.tensor_tensor(
                            out=S[:, j * P:(j + 1) * P],
                            in0=dr_sb[:, t * T + j:t * T + j + 1].to_broadcast([P, P]),
                            in1=iota_sb[:, :P],
                            op=mybir.AluOpType.is_equal,
                        )
                agg_ps = psA.tile([P, P], dt.float32)
                for j in range(T):
                    nc.tensor.matmul(
                        out=agg_ps[:],
                        lhsT=S[:, j * P:(j + 1) * P],
                        rhs=msgs[:, j, :],
                        start=(j == 0),
                        stop=(j == T - 1),
                    )
                agg_sb = work.tile([P, P], dt.bfloat16)
                nc.scalar.mul(agg_sb[:], agg_ps[:], recip[:, t:t + 1])
                aggT_ps = psB.tile([P, P], dt.bfloat16)
                nc.tensor.transpose(out=aggT_ps[:], in_=agg_sb[:], identity=ident[:])
                aggT = work.tile([P, P], dt.bfloat16)
                nc.vector.tensor_copy(aggT[:], aggT_ps[:])
                h_sb = []
                for so in range(2):
                    z_ps = psC.tile([P, P], dt.float32)
                    nc.tensor.matmul(out=z_ps[:], lhsT=wl_sb[:, so * P:(so + 1) * P],
                                     rhs=aggT[:], start=True, stop=False)
                    nc.tensor.matmul(out=z_ps[:], lhsT=wr_sb[:, so * P:(so + 1) * P],
                                     rhs=self_sb[:, t * P:(t + 1) * P], start=False, stop=True)
                    hso = outp.tile([P, P], dt.bfloat16)
                    nc.scalar.activation(hso[:], z_ps[:], mybir.ActivationFunctionType.Relu,
                                         bias=b1_sb[:, so:so + 1], scale=1.0)
                    nc.sync.dma_start(hT[so * P:(so + 1) * P, t * P:(t + 1) * P], hso[:])
                    h_sb.append(hso)
                pT_ps = psD.tile([DIM_OUT, P], dt.float32)
                for si in range(2):
                    nc.tensor.matmul(out=pT_ps[:], lhsT=w2l_sb[:, si * DIM_OUT:(si + 1) * DIM_OUT],
                                     rhs=h_sb[si][:], start=(si == 0), stop=(si == 1))
                pT_sb = outp.tile([DIM_OUT, P], dt.bfloat16)
                nc.vector.tensor_copy(pT_sb[:], pT_ps[:])
                nc.sync.dma_start(pT[:, t * P:(t + 1) * P], pT_sb[:])
    nc.compile()
    return nc


def _build_layer2(T, bounds):
    """Layer 2: mean-aggregate(p) + h @ W2r^T + b2 as an SPMD bass program."""
    NG = len(bounds)
    nc = bacc.Bacc("TRN2", target_bir_lowering=False, debug=False,
                   enable_asserts=False, num_devices=N_CORES,
                   dynamic_dma_scratch_size=65536)
    dt = mybir.dt
    tbls = [nc.dram_tensor(f"tbl{g}", [TBL_ROWS, P], dt.bfloat16, kind="ExternalInput").ap()
            for g in range(NG)]
    idx = nc.dram_tensor("idx", [P, TILES_PER_CORE * P], dt.int16, kind="ExternalInput").ap()
    dst_rel = nc.dram_tensor("dst_rel", [P, TILES_PER_CORE * T], dt.bfloat16, kind="ExternalInput").ap()
    deg_col = nc.dram_tensor("deg_col", [P, TILES_PER_CORE], dt.float32, kind="ExternalInput").ap()
    hT0 = nc.dram_tensor("hT0", [P, NPAD_CORE], dt.bfloat16, kind="ExternalInput").ap()
    hT1 = nc.dram_tensor("hT1", [P, NPAD_CORE], dt.bfloat16, kind="ExternalInput").ap()
    w2rT = nc.dram_tensor("w2rT", [P, DIM_OUT * 2], dt.bfloat16, kind="ExternalInput").ap()
    b2row = nc.dram_tensor("b2row", [1, DIM_OUT], dt.bfloat16, kind="ExternalInput").ap()
    ones = nc.dram_tensor("ones", [1, P], dt.bfloat16, kind="ExternalInput").ap()
    iota = nc.dram_tensor("iota", [P, T * P], dt.bfloat16, kind="ExternalInput").ap()
    out = nc.dram_tensor("out", [NPAD_CORE, DIM_OUT], dt.float32, kind="ExternalOutput").ap()

    with tile.TileContext(nc) as tc:
        with ExitStack() as ctx:
            const = ctx.enter_context(tc.tile_pool(name="const", bufs=1))
            msgp = ctx.enter_context(tc.tile_pool(name="msgp", bufs=2))
            sp = ctx.enter_context(tc.tile_pool(name="sp", bufs=2))
            work = ctx.enter_context(tc.tile_pool(name="work", bufs=2))
            outp = ctx.enter_context(tc.tile_pool(name="outp", bufs=3))
            psA = ctx.enter_context(tc.tile_pool(name="psA", bufs=2, space="PSUM"))
            psB = ctx.enter_context(tc.tile_pool(name="psB", bufs=2, space="PSUM"))

            warm_idx = const.tile([P, 8], dt.int16)
            nc.vector.memset(warm_idx[:], 0)
            warm_out = const.tile([P, 1, P], dt.bfloat16)
            nc.gpsimd.dma_gather(
                out_ap=warm_out[:], in_ap=tbls[0][:, :], idxs_ap=warm_idx[:],
                num_idxs=P, num_idxs_reg=P, elem_size=P,
            )
            idx_sb = const.tile([P, TILES_PER_CORE * P], dt.int16)
            nc.sync.dma_start(idx_sb[:], idx[:, :])
            dr_sb = const.tile([P, TILES_PER_CORE * T], dt.bfloat16)
            nc.sync.dma_start(dr_sb[:], dst_rel[:, :])
            deg_sb = const.tile([P, TILES_PER_CORE], dt.float32)
            nc.sync.dma_start(deg_sb[:], deg_col[:, :])
            h0_sb = const.tile([P, NPAD_CORE], dt.bfloat16)
            nc.sync.dma_start(h0_sb[:], hT0[:, :])
            h1_sb = const.tile([P, NPAD_CORE], dt.bfloat16)
            nc.sync.dma_start(h1_sb[:], hT1[:, :])
            w2r_sb = const.tile([P, DIM_OUT * 2], dt.bfloat16)
            nc.sync.dma_start(w2r_sb[:], w2rT[:, :])
            b2_sb = const.tile([1, DIM_OUT], dt.bfloat16)
            nc.sync.dma_start(b2_sb[:], b2row[:, :])
            ones_sb = const.tile([1, P], dt.bfloat16)
            nc.sync.dma_start(ones_sb[:], ones[:, :])
            iota_sb = const.tile([P, T * P], dt.bfloat16)
            nc.sync.dma_start(iota_sb[:], iota[:, :])

            recip = const.tile([P, TILES_PER_CORE], dt.float32)
            nc.vector.tensor_scalar_max(recip[:], deg_sb[:], 1.0)
            nc.vector.reciprocal(recip[:], recip[:])

            for t in range(TILES_PER_CORE):
                g = next(i for i, (t0, t1) in enumerate(bounds) if t0 <= t < t1)
                msgs = msgp.tile([P, T, P], dt.bfloat16)
                for q in range(0, T, GCH):
                    qe = min(q + GCH, T)
                    nc.gpsimd.dma_gather(
                        out_ap=msgs[:, q:qe, :],
                        in_ap=tbls[g][:, :],
                        idxs_ap=idx_sb[:, t * P + q * (P // 16):t * P + qe * (P // 16)],
                        num_idxs=(qe - q) * P,
                        num_idxs_reg=(qe - q) * P,
                        elem_size=P,
                    )
                S = sp.tile([P, T * P], dt.bfloat16)
                try:
                    nc.vector.tensor_tensor(
                        out=S[:],
                        in0=dr_sb[:, t * T:(t + 1) * T, None].to_broadcast([P, T, P]),
                        in1=iota_sb[:],
                        op=mybir.AluOpType.is_equal,
                    )
                except Exception:
                    for j in range(T):
                        nc.vector.tensor_tensor(
                            out=S[:, j * P:(j + 1) * P],
                            in0=dr_sb[:, t * T + j:t * T + j + 1].to_broadcast([P, P]),
                            in1=iota_sb[:, :P],
                            op=mybir.AluOpType.is_equal,
                        )
                agg_ps = psA.tile([P, DIM_OUT], dt.float32)
                for j in range(T):
                    nc.tensor.matmul(
                        out=agg_ps[:],
                        lhsT=S[:, j * P:(j + 1) * P],
                        rhs=msgs[:, j, :DIM_OUT],
                        start=(j == 0),
                        stop=(j == T - 1),
                    )
                agg_sb = work.tile([P, DIM_OUT], dt.float32)
                nc.scalar.mul(agg_sb[:], agg_ps[:], recip[:, t:t + 1])
                z_ps = psB.tile([P, DIM_OUT], dt.float32)
                nc.tensor.matmul(out=z_ps[:], lhsT=h0_sb[:, t * P:(t + 1) * P],
                                 rhs=w2r_sb[:, :DIM_OUT], start=True, stop=False)
                nc.tensor.matmul(out=z_ps[:], lhsT=h1_sb[:, t * P:(t + 1) * P],
                                 rhs=w2r_sb[:, DIM_OUT:], start=False, stop=False)
                nc.tensor.matmul(out=z_ps[:], lhsT=ones_sb[:, :],
                                 rhs=b2_sb[:, :], start=False, stop=True)
                o_sb = outp.tile([P, DIM_OUT], dt.float32)
                nc.vector.tensor_add(o_sb[:], z_ps[:], agg_sb[:])
                nc.sync.dma_start(out[t * P:(t + 1) * P, :], o_sb[:])
    nc.compile()
    return nc


_PROG_CACHE = {}


def _get_programs(T, bounds):
    key = (T, tuple(bounds))
    if key not in _PROG_CACHE:
        l1 = _build_layer1(T, bounds)
        l2 = _build_layer2(T, bounds)
        _PROG_CACHE[key] = (l1, l2)
    return _PROG_CACHE[key]


def kernel(x, edge_index, W1l, W1r, b1, W2l, W2r, b2):
    global LAST_RESULTS
    LAST_RESULTS = []
    x = np.asarray(x, np.float32)
    src = np.asarray(edge_index[0], np.int64)
    dst = np.asarray(edge_index[1], np.int64)

    deg = np.bincount(dst, minlength=N_NODES)
    tile_of, slot_of, T = _partition_nodes(deg)
    src_cols, dst_cols = _build_edge_layout(src, dst, tile_of, slot_of, T)
    bounds = _pick_groups(src_cols, dst_cols, T)
    uniqs, idx16 = _build_gather_meta(src_cols, dst_cols, T, bounds)
    NG = len(bounds)

    pos_of = tile_of * P + slot_of        # global padded slot (core = tile//49)
    l1, l2 = _get_programs(T, bounds)

    trace = bool(int(__import__("os").environ.get("BASS_TRACE", "0") or 0))
    tkw = dict(trace=True, tmpdir=None) if trace else {}

    x_bf = x.astype(BF16)
    iota_np = np.tile(np.arange(P, dtype=np.float32), (P, T)).astype(BF16)

    # per-core metadata
    deg_cols, selfTs, x_tbls = [], [], []
    for c in range(N_CORES):
        sl = slice(c * TILES_PER_CORE, (c + 1) * TILES_PER_CORE)
        dcol = np.zeros((P, TILES_PER_CORE), np.float32)
        sT = np.zeros((NPAD_CORE, DIM_IN), BF16)
        tiles = np.arange(*sl.indices(N_TILES)[:2])
        mask = np.isin(tile_of, tiles)
        nodes = np.nonzero(mask)[0]
        local = (tile_of[nodes] - c * TILES_PER_CORE) * P + slot_of[nodes]
        dcol[slot_of[nodes], tile_of[nodes] - c * TILES_PER_CORE] = deg[nodes]
        sT[local] = x_bf[nodes]
        deg_cols.append(dcol)
        selfTs.append(np.ascontiguousarray(sT.T))
        tbls = []
        for g in range(NG):
            tb = np.zeros((TBL_ROWS, P), BF16)
            tb[:len(uniqs[c][g])] = x_bf[uniqs[c][g]]
            tbls.append(tb)
        x_tbls.append(tbls)

    W1l, W1r, W2l, W2r = (np.asarray(a, np.float32) for a in (W1l, W1r, W2l, W2r))
    wl_p = np.ascontiguousarray(W1l.T).astype(BF16)            # [128, 256]
    wr_p = np.ascontiguousarray(W1r.T).astype(BF16)
    b1_p = np.zeros((P, 2), np.float32)
    b1_p[:, 0] = np.asarray(b1, np.float32)[:P]
    b1_p[:, 1] = np.asarray(b1, np.float32)[P:]
    # w2lT[p, si*64+o] = W2l[o, si*128+p]
    w2l_p = np.concatenate([W2l[:, si * P:(si + 1) * P].T for si in range(2)], axis=1).astype(BF16)
    w2r_p = np.concatenate([W2r[:, si * P:(si + 1) * P].T for si in range(2)], axis=1).astype(BF16)
    b2_p = np.asarray(b2, np.float32).reshape(1, DIM_OUT).astype(BF16)
    ones_p = np.ones((1, P), BF16)

    in_maps = []
    for c in range(N_CORES):
        m = {f"tbl{g}": x_tbls[c][g] for g in range(NG)}
        m.update({
            "idx": idx16[c],
            "dst_rel": dst_cols[c].astype(BF16),
            "deg_col": deg_cols[c],
            "selfT": selfTs[c],
            "wl": wl_p, "wr": wr_p, "b1": b1_p, "w2lT": w2l_p,
            "iota": iota_np, "identity": np.eye(P, dtype=BF16),
        })
        in_maps.append(m)
    r1 = _run_spmd_retry(l1, in_maps, **tkw)
    LAST_RESULTS.append(r1)

    # assemble p gather tables: p rows are indexed by global padded position
    pT_all = np.concatenate([np.asarray(r1.results[c]["pT"]) for c in range(N_CORES)],
                            axis=1)                             # [64, 50176] bf16
    p_rows = np.ascontiguousarray(pT_all.T)                     # [50176, 64]

    in_maps2 = []
    for c in range(N_CORES):
        m = {}
        for g in range(NG):
            tb = np.zeros((TBL_ROWS, P), BF16)
            tb[:len(uniqs[c][g]), :DIM_OUT] = p_rows[pos_of[uniqs[c][g]]]
            m[f"tbl{g}"] = tb
        hT = np.asarray(r1.results[c]["hT"])                    # [256, 6272] bf16
        m.update({
            "idx": idx16[c],
            "dst_rel": dst_cols[c].astype(BF16),
            "deg_col": deg_cols[c],
            "hT0": np.ascontiguousarray(hT[:P]),
            "hT1": np.ascontiguousarray(hT[P:]),
            "w2rT": w2r_p, "b2row": b2_p, "ones": ones_p,
            "iota": iota_np,
        })
        in_maps2.append(m)
    r2 = _run_spmd_retry(l2, in_maps2, **tkw)
    LAST_RESULTS.append(r2)

    big = np.concatenate([np.asarray(r2.results[c]["out"]) for c in range(N_CORES)],
                         axis=0)                                # [50176, 64] f32
    out = np.ascontiguousarray(big[pos_of[np.arange(N_NODES)]], dtype=np.float32)
    return out


# revision 12
# speedup vs baseline: 1.0015x; 1.0015x over previous
"""GraphSAGE (2-layer, mean aggregation) on 8 Trainium2 NeuronCores.

Strategy: destination nodes are sharded across the 8 cores (49 tiles of 128
nodes per core, LPT-balanced by degree). Edges are partitioned by destination
tile, padded to a uniform chunk count T per tile so one SPMD program serves
all cores.

Per-edge source rows are fetched with SWDGE `dma_gather` (two 1024-index
instructions per tile — 1024 descriptors is the per-instruction SWDGE scratch
cap; larger gathers fault the device). dma_gather uses int16 indices (max
32767), so each core's 49 tiles are split into index groups whose unique
source-node sets fit in a 32768-row gather table; tables hold bf16 feature
rows (256 B, the SWDGE minimum element). Measured Q7 descriptor generation
runs ~9 ns/row serially per core, which makes the gathers the span-defining
cost of both layers; the PE/DVE/Scalar work and the DMA transfers themselves
all hide underneath.

The segment sum for a destination tile runs on the PE in bf16: a 0/1
selection matrix S[e, n] = (dst_slot[e] == n) is formed on the vector engine
(iota compare) and S^T @ messages accumulates into PSUM over the tile's
chunks. Mean division, dense lin_l/lin_r matmuls, bias and ReLU happen
on-device in bf16 (fp32 PSUM accumulate).

Layer 2 exploits linearity: p = h @ W2l^T ([N, 64]) is computed at the end of
layer 1 (per-core, own nodes), so layer-2 messages are 64-wide instead of
256-wide — 4x less gather traffic and PE work. Layer 2 then only needs
mean-aggregate(p) + h @ W2r^T + b2, with the bias added via a K=1 matmul
(ones ⊗ b2) into the same PSUM accumulation.

The host does integer index preprocessing, sharding/layout, bf16 casts and
un-sharding; all float tensor math runs on the NeuronCores.
"""
import heapq
import sys
from contextlib import ExitStack

import numpy as np
import ml_dtypes

for _p in ("/opt/trn_rl_repo",):
    if _p not in sys.path:
        sys.path.insert(0, _p)

import concourse.tile as tile
from concourse import bacc, mybir
from concourse.bass_utils import run_bass_kernel_spmd

BF16 = ml_dtypes.bfloat16


def _ensure_axon_hooks():
    """run_bass_kernel_spmd(trace=True) imports antenv.axon_hooks, which this
    image lacks; install a ctypes-backed hook so tracing works (or degrades
    to a no-op instead of an ImportError)."""
    try:
        import antenv.axon_hooks  # noqa: F401
        return
    except ImportError:
        pass
    import contextlib
    import ctypes
    import types

    def _make_hook():
        try:
            lib = ctypes.CDLL("/opt/axon/libaxon_pjrt.so")
        except OSError:
            return None
        if not hasattr(lib, "axon_start_nrt_profile"):
            return None
        lib.axon_start_nrt_profile.argtypes = [ctypes.POINTER(ctypes.c_int64), ctypes.c_size_t]
        lib.axon_start_nrt_profile.restype = ctypes.c_int64
        lib.axon_stop_nrt_profile.argtypes = [ctypes.c_char_p]
        lib.axon_stop_nrt_profile.restype = ctypes.c_int64

        @contextlib.contextmanager
        def _hook(output_dir, device_ids):
            import jax
            jax.devices()
            if device_ids:
                ids = (ctypes.c_int64 * len(device_ids))(*device_ids)
                rc = lib.axon_start_nrt_profile(ids, len(device_ids))
            else:
                rc = lib.axon_start_nrt_profile(None, 0)
            if rc != 0:
                raise RuntimeError(f"axon_start_nrt_profile rc={rc}")
            try:
                yield
            finally:
                lib.axon_stop_nrt_profile(str(output_dir).encode())

        return _hook

    hook = _make_hook()
    mod = types.ModuleType("antenv.axon_hooks")
    mod.get_axon_ntff_profile_hook = lambda: hook
    mod.set_axon_ntff_profile_hook = lambda h: None
    import antenv
    antenv.axon_hooks = mod
    sys.modules["antenv.axon_hooks"] = mod


_ensure_axon_hooks()


def _run_spmd_retry(nc, in_maps, **kw):
    """Retries for transient NRT device errors (axon cores report
    EXEC_UNIT_UNRECOVERABLE for ~60-120 s after a prior faulted run)."""
    import time
    for wait in (75, 120):
        try:
            return run_bass_kernel_spmd(nc, in_maps, core_ids=list(range(N_CORES)), **kw)
        except Exception:
            time.sleep(wait)
    return run_bass_kernel_spmd(nc, in_maps, core_ids=list(range(N_CORES)), **kw)

N_NODES = 50000
N_EDGES = 800000
DIM_IN, DIM_H, DIM_OUT = 128, 256, 64
N_CORES = 8
P = 128
TILES_PER_CORE = 49                      # ceil(50000 / 8 / 128)
N_TILES = N_CORES * TILES_PER_CORE       # 392
NPAD_CORE = TILES_PER_CORE * P           # 6272
PAD_SLOT = 200.0                         # dst_rel sentinel: matches no iota lane
TBL_ROWS = 32768                         # int16 gather-table row limit
GCH = 8                                  # chunks per dma_gather (<=1024 descs/inst)

LAST_RESULTS = []   # test harness reads profiling results from here


def _partition_nodes(deg):
    """LPT-pack nodes into N_TILES bins of <=128 nodes, minimizing max bin
    degree-sum. Returns (tile_of, slot_of, T) with T = uniform chunks/tile."""
    order = np.argsort(-deg, kind="stable")
    heap = [(0, t) for t in range(N_TILES)]
    heapq.heapify(heap)
    counts = np.zeros(N_TILES, np.int64)
    sums = np.zeros(N_TILES, np.int64)
    tile_of = np.empty(N_NODES, np.int64)
    slot_of = np.empty(N_NODES, np.int64)
    for node in order:
        while True:
            s, t = heapq.heappop(heap)
            if counts[t] < P:
                break
        tile_of[node] = t
        slot_of[node] = counts[t]
        counts[t] += 1
        sums[t] += deg[node]
        if counts[t] < P:
            heapq.heappush(heap, (sums[t], t))
    T = int(np.ceil(sums.max() / P))
    return tile_of, slot_of, T


def _build_edge_layout(src, dst, tile_of, slot_of, T):
    """Per-core chunk-major index arrays.

    Returns src_cols, dst_cols: lists (per core) of [P, 49*T] arrays where
    column t*T + j holds chunk j of tile t: lane p is edge j*128+p of that
    tile's padded edge list (src node id / dst slot, PAD entries src=0,
    dst_rel=PAD_SLOT).
    """
    etile = tile_of[dst]
    order = np.argsort(etile, kind="stable")
    counts = np.bincount(etile, minlength=N_TILES)
    src_pad = np.zeros((N_TILES, T * P), np.int64)
    dst_pad = np.full((N_TILES, T * P), PAD_SLOT, np.float32)
    rank = np.arange(N_EDGES) - np.repeat(np.concatenate([[0], np.cumsum(counts)[:-1]]), counts)
    es, ed = src[order], dst[order]
    src_pad[etile[order], rank] = es
    dst_pad[etile[order], rank] = slot_of[ed]
    src_cols, dst_cols = [], []
    for c in range(N_CORES):
        sl = slice(c * TILES_PER_CORE, (c + 1) * TILES_PER_CORE)
        s = src_pad[sl].reshape(TILES_PER_CORE, T, P).transpose(2, 0, 1).reshape(P, TILES_PER_CORE * T)
        d = dst_pad[sl].reshape(TILES_PER_CORE, T, P).transpose(2, 0, 1).reshape(P, TILES_PER_CORE * T)
        src_cols.append(np.ascontiguousarray(s))
        dst_cols.append(np.ascontiguousarray(d))
    return src_cols, dst_cols


def _pick_groups(src_cols, dst_cols, T):
    """Split each core's 49 tiles into contiguous groups whose unique
    source sets fit a 32768-row table. Greedy over tiles, max over cores so
    group bounds are uniform (SPMD). Returns list of (start_tile, end_tile)."""
    bounds = []
    start = 0
    while start < TILES_PER_CORE:
        end = start + 1
        while end < TILES_PER_CORE:
            ok = True
            for c in range(N_CORES):
                blk = src_cols[c][:, start * T:(end + 1) * T]
                pad = dst_cols[c][:, start * T:(end + 1) * T] == PAD_SLOT
                n_uniq = len(np.unique(blk[~pad]))
                if n_uniq > TBL_ROWS - 8:
                    ok = False
                    break
            if not ok:
                break
            end += 1
        bounds.append((start, end))
        start = end
    return bounds


def _build_gather_meta(src_cols, dst_cols, T, bounds):
    """Per (core, group): unique source node list + int16 index array.

    Returns uniqs[c][g] (node ids) and idx16[c] [P, 49*P] int16 where tile
    t's block [:, t*P:(t+1)*P] holds the dma_gather index layout: edge
    i = j*128+p of the tile maps to [i % 16 (replicated x8), t*P + i//16].
    """
    uniqs = [[None] * len(bounds) for _ in range(N_CORES)]
    idx16 = []
    for c in range(N_CORES):
        out = np.zeros((P, TILES_PER_CORE * P), np.int16)
        for g, (t0, t1) in enumerate(bounds):
            blk = src_cols[c][:, t0 * T:t1 * T]
            pad = dst_cols[c][:, t0 * T:t1 * T] == PAD_SLOT
            uniq = np.unique(blk[~pad]) if (~pad).any() else np.array([0], np.int64)
            assert len(uniq) <= TBL_ROWS, f"group {g} core {c}: {len(uniq)} uniques"
            remap = np.zeros(N_NODES, np.int64)
            remap[uniq] = np.arange(len(uniq))
            uniqs[c][g] = uniq
            for t in range(t0, t1):
                arr = src_cols[c][:, t * T:(t + 1) * T]          # [P, T] lane p chunk j
                padm = dst_cols[c][:, t * T:(t + 1) * T] == PAD_SLOT
                r = remap[arr]
                r[padm] = 0
                flat = r.T.reshape(-1)                            # i = j*128+p order
                blk16 = flat.reshape(T * P // 16, 16).T           # [16, T*P/16]
                out[:, t * P:(t + 1) * P] = np.tile(blk16, (8, 1)).astype(np.int16)
        idx16.append(out)
    return uniqs, idx16


def _build_layer1(T, bounds):
    """Layer 1 + p-pretransform as an SPMD bass program."""
    NG = len(bounds)
    nc = bacc.Bacc("TRN2", target_bir_lowering=False, debug=False,
                   enable_asserts=False, num_devices=N_CORES,
                   dynamic_dma_scratch_size=49152)
    dt = mybir.dt
    tbls = [nc.dram_tensor(f"tbl{g}", [TBL_ROWS, P], dt.bfloat16, kind="ExternalInput").ap()
            for g in range(NG)]
    idx = nc.dram_tensor("idx", [P, TILES_PER_CORE * P], dt.int16, kind="ExternalInput").ap()
    dst_rel = nc.dram_tensor("dst_rel", [P, TILES_PER_CORE * T], dt.bfloat16, kind="ExternalInput").ap()
    deg_col = nc.dram_tensor("deg_col", [P, TILES_PER_CORE], dt.float32, kind="ExternalInput").ap()
    selfT = nc.dram_tensor("selfT", [P, NPAD_CORE], dt.bfloat16, kind="ExternalInput").ap()
    wl = nc.dram_tensor("wl", [P, DIM_H], dt.bfloat16, kind="ExternalInput").ap()
    wr = nc.dram_tensor("wr", [P, DIM_H], dt.bfloat16, kind="ExternalInput").ap()
    b1 = nc.dram_tensor("b1", [P, 2], dt.float32, kind="ExternalInput").ap()
    w2lT = nc.dram_tensor("w2lT", [P, DIM_OUT * 2], dt.bfloat16, kind="ExternalInput").ap()
    iota = nc.dram_tensor("iota", [P, T * P], dt.bfloat16, kind="ExternalInput").ap()
    identity = nc.dram_tensor("identity", [P, P], dt.bfloat16, kind="ExternalInput").ap()
    hT = nc.dram_tensor("hT", [DIM_H, NPAD_CORE], dt.bfloat16, kind="ExternalOutput").ap()
    pT = nc.dram_tensor("pT", [DIM_OUT, NPAD_CORE], dt.bfloat16, kind="ExternalOutput").ap()

    with tile.TileContext(nc) as tc:
        with ExitStack() as ctx:
            const = ctx.enter_context(tc.tile_pool(name="const", bufs=1))
            msgp = ctx.enter_context(tc.tile_pool(name="msgp", bufs=2))
            sp = ctx.enter_context(tc.tile_pool(name="sp", bufs=2))
            work = ctx.enter_context(tc.tile_pool(name="work", bufs=2))
            outp = ctx.enter_context(tc.tile_pool(name="outp", bufs=3))
            psA = ctx.enter_context(tc.tile_pool(name="psA", bufs=2, space="PSUM"))
            psB = ctx.enter_context(tc.tile_pool(name="psB", bufs=2, space="PSUM"))
            psC = ctx.enter_context(tc.tile_pool(name="psC", bufs=2, space="PSUM"))
            psD = ctx.enter_context(tc.tile_pool(name="psD", bufs=2, space="PSUM"))

            idx_sb = const.tile([P, TILES_PER_CORE * P], dt.int16)
            nc.sync.dma_start(idx_sb[:], idx[:, :])
            dr_sb = const.tile([P, TILES_PER_CORE * T], dt.bfloat16)
            nc.sync.dma_start(dr_sb[:], dst_rel[:, :])
            deg_sb = const.tile([P, TILES_PER_CORE], dt.float32)
            nc.sync.dma_start(deg_sb[:], deg_col[:, :])
            self_sb = const.tile([P, NPAD_CORE], dt.bfloat16)
            nc.sync.dma_start(self_sb[:], selfT[:, :])
            wl_sb = const.tile([P, DIM_H], dt.bfloat16)
            nc.sync.dma_start(wl_sb[:], wl[:, :])
            wr_sb = const.tile([P, DIM_H], dt.bfloat16)
            nc.sync.dma_start(wr_sb[:], wr[:, :])
            b1_sb = const.tile([P, 2], dt.float32)
            nc.sync.dma_start(b1_sb[:], b1[:, :])
            w2l_sb = const.tile([P, DIM_OUT * 2], dt.bfloat16)
            nc.sync.dma_start(w2l_sb[:], w2lT[:, :])
            iota_sb = const.tile([P, T * P], dt.bfloat16)
            nc.sync.dma_start(iota_sb[:], iota[:, :])
            ident = const.tile([P, P], dt.bfloat16)
            nc.sync.dma_start(ident[:], identity[:, :])

            recip = const.tile([P, TILES_PER_CORE], dt.float32)
            nc.vector.tensor_scalar_max(recip[:], deg_sb[:], 1.0)
            nc.vector.reciprocal(recip[:], recip[:])

            for t in range(TILES_PER_CORE):
                g = next(i for i, (t0, t1) in enumerate(bounds) if t0 <= t < t1)
                msgs = msgp.tile([P, T, P], dt.bfloat16)
                for q in range(0, T, GCH):
                    qe = min(q + GCH, T)
                    nc.gpsimd.dma_gather(
                        out_ap=msgs[:, q:qe, :],
                        in_ap=tbls[g][:, :],
                        idxs_ap=idx_sb[:, t * P + q * (P // 16):t * P + qe * (P // 16)],
                        num_idxs=(qe - q) * P,
                        num_idxs_reg=(qe - q) * P,
                        elem_size=P,
                    )
                S = sp.tile([P, T * P], dt.bfloat16)
                try:
                    nc.vector.tensor_tensor(
                        out=S[:],
                        in0=dr_sb[:, t * T:(t + 1) * T, None].to_broadcast([P, T, P]),
                        in1=iota_sb[:],
                        op=mybir.AluOpType.is_equal,
                    )
                except Exception:
                    for j in range(T):
                        nc.vector.tensor_tensor(
                            out=S[:, j * P:(j + 1) * P],
                            in0=dr_sb[:, t * T + j:t * T + j + 1].to_broadcast([P, P]),
                            in1=iota_sb[:, :P],
                            op=mybir.AluOpType.is_equal,
                        )
                agg_ps = psA.tile([P, P], dt.float32)
                for j in range(T):
                    nc.tensor.matmul(
                        out=agg_ps[:],
                        lhsT=S[:, j * P:(j + 1) * P],
                        rhs=msgs[:, j, :],
                        start=(j == 0),
                        stop=(j == T - 1),
                    )
                agg_sb = work.tile([P, P], dt.bfloat16)
                nc.scalar.mul(agg_sb[:], agg_ps[:], recip[:, t:t + 1])
                aggT_ps = psB.tile([P, P], dt.bfloat16)
                nc.tensor.transpose(out=aggT_ps[:], in_=agg_sb[:], identity=ident[:])
                aggT = work.tile([P, P], dt.bfloat16)
                nc.vector.tensor_copy(aggT[:], aggT_ps[:])
                h_sb = []
                for so in range(2):
                    z_ps = psC.tile([P, P], dt.float32)
                    nc.tensor.matmul(out=z_ps[:], lhsT=wl_sb[:, so * P:(so + 1) * P],
                                     rhs=aggT[:], start=True, stop=False)
                    nc.tensor.matmul(out=z_ps[:], lhsT=wr_sb[:, so * P:(so + 1) * P],
                                     rhs=self_sb[:, t * P:(t + 1) * P], start=False, stop=True)
                    hso = outp.tile([P, P], dt.bfloat16)
                    nc.scalar.activation(hso[:], z_ps[:], mybir.ActivationFunctionType.Relu,
                                         bias=b1_sb[:, so:so + 1], scale=1.0)
                    nc.sync.dma_start(hT[so * P:(so + 1) * P, t * P:(t + 1) * P], hso[:])
                    h_sb.append(hso)
                pT_ps = psD.tile([DIM_OUT, P], dt.float32)
                for si in range(2):
                    nc.tensor.matmul(out=pT_ps[:], lhsT=w2l_sb[:, si * DIM_OUT:(si + 1) * DIM_OUT],
                                     rhs=h_sb[si][:], start=(si == 0), stop=(si == 1))
                pT_sb = outp.tile([DIM_OUT, P], dt.bfloat16)
                nc.vector.tensor_copy(pT_sb[:], pT_ps[:])
                nc.sync.dma_start(pT[:, t * P:(t + 1) * P], pT_sb[:])
    nc.compile()
    return nc


def _build_layer2(T, bounds):
    """Layer 2: mean-aggregate(p) + h @ W2r^T + b2 as an SPMD bass program."""
    NG = len(bounds)
    nc = bacc.Bacc("TRN2", target_bir_lowering=False, debug=False,
                   enable_asserts=False, num_devices=N_CORES,
                   dynamic_dma_scratch_size=49152)
    dt = mybir.dt
    tbls = [nc.dram_tensor(f"tbl{g}", [TBL_ROWS, P], dt.bfloat16, kind="ExternalInput").ap()
            for g in range(NG)]
    idx = nc.dram_tensor("idx", [P, TILES_PER_CORE * P], dt.int16, kind="ExternalInput").ap()
    dst_rel = nc.dram_tensor("dst_rel", [P, TILES_PER_CORE * T], dt.bfloat16, kind="ExternalInput").ap()
    deg_col = nc.dram_tensor("deg_col", [P, TILES_PER_CORE], dt.float32, kind="ExternalInput").ap()
    hT0 = nc.dram_tensor("hT0", [P, NPAD_CORE], dt.bfloat16, kind="ExternalInput").ap()
    hT1 = nc.dram_tensor("hT1", [P, NPAD_CORE], dt.bfloat16, kind="ExternalInput").ap()
    w2rT = nc.dram_tensor("w2rT", [P, DIM_OUT * 2], dt.bfloat16, kind="ExternalInput").ap()
    b2row = nc.dram_tensor("b2row", [1, DIM_OUT], dt.bfloat16, kind="ExternalInput").ap()
    ones = nc.dram_tensor("ones", [1, P], dt.bfloat16, kind="ExternalInput").ap()
    iota = nc.dram_tensor("iota", [P, T * P], dt.bfloat16, kind="ExternalInput").ap()
    out = nc.dram_tensor("out", [NPAD_CORE, DIM_OUT], dt.float32, kind="ExternalOutput").ap()

    with tile.TileContext(nc) as tc:
        with ExitStack() as ctx:
            const = ctx.enter_context(tc.tile_pool(name="const", bufs=1))
            msgp = ctx.enter_context(tc.tile_pool(name="msgp", bufs=2))
            sp = ctx.enter_context(tc.tile_pool(name="sp", bufs=2))
            work = ctx.enter_context(tc.tile_pool(name="work", bufs=2))
            outp = ctx.enter_context(tc.tile_pool(name="outp", bufs=3))
            psA = ctx.enter_context(tc.tile_pool(name="psA", bufs=2, space="PSUM"))
            psB = ctx.enter_context(tc.tile_pool(name="psB", bufs=2, space="PSUM"))

            idx_sb = const.tile([P, TILES_PER_CORE * P], dt.int16)
            nc.sync.dma_start(idx_sb[:], idx[:, :])
            dr_sb = const.tile([P, TILES_PER_CORE * T], dt.bfloat16)
            nc.sync.dma_start(dr_sb[:], dst_rel[:, :])
            deg_sb = const.tile([P, TILES_PER_CORE], dt.float32)
            nc.sync.dma_start(deg_sb[:], deg_col[:, :])
            h0_sb = const.tile([P, NPAD_CORE], dt.bfloat16)
            nc.sync.dma_start(h0_sb[:], hT0[:, :])
            h1_sb = const.tile([P, NPAD_CORE], dt.bfloat16)
            nc.sync.dma_start(h1_sb[:], hT1[:, :])
            w2r_sb = const.tile([P, DIM_OUT * 2], dt.bfloat16)
            nc.sync.dma_start(w2r_sb[:], w2rT[:, :])
            b2_sb = const.tile([1, DIM_OUT], dt.bfloat16)
            nc.sync.dma_start(b2_sb[:], b2row[:, :])
            ones_sb = const.tile([1, P], dt.bfloat16)
            nc.sync.dma_start(ones_sb[:], ones[:, :])
            iota_sb = const.tile([P, T * P], dt.bfloat16)
            nc.sync.dma_start(iota_sb[:], iota[:, :])

            recip = const.tile([P, TILES_PER_CORE], dt.float32)
            nc.vector.tensor_scalar_max(recip[:], deg_sb[:], 1.0)
            nc.vector.reciprocal(recip[:], recip[:])

            for t in range(TILES_PER_CORE):
                g = next(i for i, (t0, t1) in enumerate(bounds) if t0 <= t < t1)
                msgs = msgp.tile([P, T, P], dt.bfloat16)
                for q in range(0, T, GCH):
                    qe = min(q + GCH, T)
                    nc.gpsimd.dma_gather(
                        out_ap=msgs[:, q:qe, :],
                        in_ap=tbls[g][:, :],
                        idxs_ap=idx_sb[:, t * P + q * (P // 16):t * P + qe * (P // 16)],
                        num_idxs=(qe - q) * P,
                        num_idxs_reg=(qe - q) * P,
                        elem_size=P,
                    )
                S = sp.tile([P, T * P], dt.bfloat16)
                try:
                    nc.vector.tensor_tensor(
                        out=S[:],
                        in0=dr_sb[:, t * T:(t + 1) * T, None].to_broadcast([P, T, P]),
                        in1=iota_sb[:],
                        op=mybir.AluOpType.is_equal,
                    )
                except Exception:
                    for j in range(T):
                        nc.vector.tensor_tensor(
                            out=S[:, j * P:(j + 1) * P],
                            in0=dr_sb[:, t * T + j:t * T + j + 1].to_broadcast([P, P]),
                            in1=iota_sb[:, :P],
                            op=mybir.AluOpType.is_equal,
                        )
                agg_ps = psA.tile([P, DIM_OUT], dt.float32)
                for j in range(T):
                    nc.tensor.matmul(
                        out=agg_ps[:],
                        lhsT=S[:, j * P:(j + 1) * P],
                        rhs=msgs[:, j, :DIM_OUT],
                        start=(j == 0),
                        stop=(j == T - 1),
                    )
                agg_sb = work.tile([P, DIM_OUT], dt.float32)
                nc.scalar.mul(agg_sb[:], agg_ps[:], recip[:, t:t + 1])
                z_ps = psB.tile([P, DIM_OUT], dt.float32)
                nc.tensor.matmul(out=z_ps[:], lhsT=h0_sb[:, t * P:(t + 1) * P],
                                 rhs=w2r_sb[:, :DIM_OUT], start=True, stop=False)
                nc.tensor.matmul(out=z_ps[:], lhsT=h1_sb[:, t * P:(t + 1) * P],
                                 rhs=w2r_sb[:, DIM_OUT:], start=False, stop=False)
                nc.tensor.matmul(out=z_ps[:], lhsT=ones_sb[:, :],
                                 rhs=b2_sb[:, :], start=False, stop=True)
                o_sb = outp.tile([P, DIM_OUT], dt.float32)
                nc.vector.tensor_add(o_sb[:], z_ps[:], agg_sb[:])
                nc.sync.dma_start(out[t * P:(t + 1) * P, :], o_sb[:])
    nc.compile()
    return nc


_PROG_CACHE = {}


def _get_programs(T, bounds):
    key = (T, tuple(bounds))
    if key not in _PROG_CACHE:
        l1 = _build_layer1(T, bounds)
        l2 = _build_layer2(T, bounds)
        _PROG_CACHE[key] = (l1, l2)
    return _PROG_CACHE[key]


def kernel(x, edge_index, W1l, W1r, b1, W2l, W2r, b2):
    global LAST_RESULTS
    LAST_RESULTS = []
    x = np.asarray(x, np.float32)
    src = np.asarray(edge_index[0], np.int64)
    dst = np.asarray(edge_index[1], np.int64)

    deg = np.bincount(dst, minlength=N_NODES)
    tile_of, slot_of, T = _partition_nodes(deg)
    src_cols, dst_cols = _build_edge_layout(src, dst, tile_of, slot_of, T)
    bounds = _pick_groups(src_cols, dst_cols, T)
    uniqs, idx16 = _build_gather_meta(src_cols, dst_cols, T, bounds)
    NG = len(bounds)

    pos_of = tile_of * P + slot_of        # global padded slot (core = tile//49)
    l1, l2 = _get_programs(T, bounds)

    trace = bool(int(__import__("os").environ.get("BASS_TRACE", "0") or 0))
    tkw = dict(trace=True, tmpdir=None) if trace else {}

    x_bf = x.astype(BF16)
    iota_np = np.tile(np.arange(P, dtype=np.float32), (P, T)).astype(BF16)

    # per-core metadata
    deg_cols, selfTs, x_tbls = [], [], []
    for c in range(N_CORES):
        sl = slice(c * TILES_PER_CORE, (c + 1) * TILES_PER_CORE)
        dcol = np.zeros((P, TILES_PER_CORE), np.float32)
        sT = np.zeros((NPAD_CORE, DIM_IN), BF16)
        tiles = np.arange(*sl.indices(N_TILES)[:2])
        mask = np.isin(tile_of, tiles)
        nodes = np.nonzero(mask)[0]
        local = (tile_of[nodes] - c * TILES_PER_CORE) * P + slot_of[nodes]
        dcol[slot_of[nodes], tile_of[nodes] - c * TILES_PER_CORE] = deg[nodes]
        sT[local] = x_bf[nodes]
        deg_cols.append(dcol)
        selfTs.append(np.ascontiguousarray(sT.T))
        tbls = []
        for g in range(NG):
            tb = np.zeros((TBL_ROWS, P), BF16)
            tb[:len(uniqs[c][g])] = x_bf[uniqs[c][g]]
            tbls.append(tb)
        x_tbls.append(tbls)

    W1l, W1r, W2l, W2r = (np.asarray(a, np.float32) for a in (W1l, W1r, W2l, W2r))
    wl_p = np.ascontiguousarray(W1l.T).astype(BF16)            # [128, 256]
    wr_p = np.ascontiguousarray(W1r.T).astype(BF16)
    b1_p = np.zeros((P, 2), np.float32)
    b1_p[:, 0] = np.asarray(b1, np.float32)[:P]
    b1_p[:, 1] = np.asarray(b1, np.float32)[P:]
    # w2lT[p, si*64+o] = W2l[o, si*128+p]
    w2l_p = np.concatenate([W2l[:, si * P:(si + 1) * P].T for si in range(2)], axis=1).astype(BF16)
    w2r_p = np.concatenate([W2r[:, si * P:(si + 1) * P].T for si in range(2)], axis=1).astype(BF16)
    b2_p = np.asarray(b2, np.float32).reshape(1, DIM_OUT).astype(BF16)
    ones_p = np.ones((1, P), BF16)

    in_maps = []
    for c in range(N_CORES):
        m = {f"tbl{g}": x_tbls[c][g] for g in range(NG)}
        m.update({
            "idx": idx16[c],
            "dst_rel": dst_cols[c].astype(BF16),
            "deg_col": deg_cols[c],
            "selfT": selfTs[c],
            "wl": wl_p, "wr": wr_p, "b1": b1_p, "w2lT": w2l_p,
            "iota": iota_np, "identity": np.eye(P, dtype=BF16),
        })
        in_maps.append(m)
    r1 = _run_spmd_retry(l1, in_maps, **tkw)
    LAST_RESULTS.append(r1)

    # assemble p gather tables: p rows are indexed by global padded position
    pT_all = np.concatenate([np.asarray(r1.results[c]["pT"]) for c in range(N_CORES)],
                            axis=1)                             # [64, 50176] bf16
    p_rows = np.ascontiguousarray(pT_all.T)                     # [50176, 64]

    in_maps2 = []
    for c in range(N_CORES):
        m = {}
        for g in range(NG):
            tb = np.zeros((TBL_ROWS, P), BF16)
            tb[:len(uniqs[c][g]), :DIM_OUT] = p_rows[pos_of[uniqs[c][g]]]
            m[f"tbl{g}"] = tb
        hT = np.asarray(r1.results[c]["hT"])                    # [256, 6272] bf16
        m.update({
            "idx": idx16[c],
            "dst_rel": dst_cols[c].astype(BF16),
            "deg_col": deg_cols[c],
            "hT0": np.ascontiguousarray(hT[:P]),
            "hT1": np.ascontiguousarray(hT[P:]),
            "w2rT": w2r_p, "b2row": b2_p, "ones": ones_p,
            "iota": iota_np,
        })
        in_maps2.append(m)
    r2 = _run_spmd_retry(l2, in_maps2, **tkw)
    LAST_RESULTS.append(r2)

    big = np.concatenate([np.asarray(r2.results[c]["out"]) for c in range(N_CORES)],
                         axis=0)                                # [50176, 64] f32
    out = np.ascontiguousarray(big[pos_of[np.arange(N_NODES)]], dtype=np.float32)
    return out


# revision 13
# speedup vs baseline: 2.2864x; 2.2831x over previous
"""GraphSAGE (2-layer, mean aggregation) on 8 Trainium2 NeuronCores.

Strategy: destination nodes are sharded across the 8 cores (49 tiles of 128
nodes per core, LPT-balanced by degree). Edges are partitioned by destination
tile, padded to a uniform chunk count T per tile so one SPMD program serves
all cores.

Per-edge source rows are fetched with SWDGE `dma_gather` (two 1024-index
instructions per tile — 1024 descriptors is the per-instruction SWDGE scratch
cap; larger gathers fault the device). dma_gather uses int16 indices (max
32767), so each core's 49 tiles are split into index groups whose unique
source-node sets fit in a 32768-row gather table; tables hold bf16 feature
rows (256 B, the SWDGE minimum element). Measured Q7 descriptor generation
runs ~9 ns/row serially per core, which makes the gathers the span-defining
cost of both layers; the PE/DVE/Scalar work and the DMA transfers themselves
all hide underneath.

The segment sum for a destination tile runs on the PE in bf16: a 0/1
selection matrix S[e, n] = (dst_slot[e] == n) is formed on the vector engine
(iota compare) and S^T @ messages accumulates into PSUM over the tile's
chunks. Mean division, dense lin_l/lin_r matmuls, bias and ReLU happen
on-device in bf16 (fp32 PSUM accumulate).

Layer 2 exploits linearity: p = h @ W2l^T ([N, 64]) is computed at the end of
layer 1 (per-core, own nodes), so layer-2 messages are 64-wide instead of
256-wide — 4x less gather traffic and PE work. Layer 2 then only needs
mean-aggregate(p) + h @ W2r^T + b2, with the bias added via a K=1 matmul
(ones ⊗ b2) into the same PSUM accumulation.

The host does integer index preprocessing, sharding/layout, bf16 casts and
un-sharding; all float tensor math runs on the NeuronCores.
"""
import heapq
import sys
from contextlib import ExitStack

import numpy as np
import ml_dtypes

for _p in ("/opt/trn_rl_repo",):
    if _p not in sys.path:
        sys.path.insert(0, _p)

import concourse.tile as tile
from concourse import bacc, mybir
from concourse.bass_utils import run_bass_kernel_spmd

BF16 = ml_dtypes.bfloat16


def _ensure_axon_hooks():
    """run_bass_kernel_spmd(trace=True) imports antenv.axon_hooks, which this
    image lacks; install a ctypes-backed hook so tracing works (or degrades
    to a no-op instead of an ImportError)."""
    try:
        import antenv.axon_hooks  # noqa: F401
        return
    except ImportError:
        pass
    import contextlib
    import ctypes
    import types

    def _make_hook():
        try:
            lib = ctypes.CDLL("/opt/axon/libaxon_pjrt.so")
        except OSError:
            return None
        if not hasattr(lib, "axon_start_nrt_profile"):
            return None
        lib.axon_start_nrt_profile.argtypes = [ctypes.POINTER(ctypes.c_int64), ctypes.c_size_t]
        lib.axon_start_nrt_profile.restype = ctypes.c_int64
        lib.axon_stop_nrt_profile.argtypes = [ctypes.c_char_p]
        lib.axon_stop_nrt_profile.restype = ctypes.c_int64

        @contextlib.contextmanager
        def _hook(output_dir, device_ids):
            import jax
            jax.devices()
            if device_ids:
                ids = (ctypes.c_int64 * len(device_ids))(*device_ids)
                rc = lib.axon_start_nrt_profile(ids, len(device_ids))
            else:
                rc = lib.axon_start_nrt_profile(None, 0)
            if rc != 0:
                raise RuntimeError(f"axon_start_nrt_profile rc={rc}")
            try:
                yield
            finally:
                lib.axon_stop_nrt_profile(str(output_dir).encode())

        return _hook

    hook = _make_hook()
    mod = types.ModuleType("antenv.axon_hooks")
    mod.get_axon_ntff_profile_hook = lambda: hook
    mod.set_axon_ntff_profile_hook = lambda h: None
    import antenv
    antenv.axon_hooks = mod
    sys.modules["antenv.axon_hooks"] = mod


_ensure_axon_hooks()


def _run_spmd_retry(nc, in_maps, **kw):
    """Retries for transient NRT device errors (axon cores report
    EXEC_UNIT_UNRECOVERABLE for ~60-120 s after a prior faulted run)."""
    import time
    for wait in (75, 120):
        try:
            return run_bass_kernel_spmd(nc, in_maps, core_ids=list(range(N_CORES)), **kw)
        except Exception:
            time.sleep(wait)
    return run_bass_kernel_spmd(nc, in_maps, core_ids=list(range(N_CORES)), **kw)

N_NODES = 50000
N_EDGES = 800000
DIM_IN, DIM_H, DIM_OUT = 128, 256, 64
N_CORES = 8
P = 128
TILES_PER_CORE = 49                      # ceil(50000 / 8 / 128)
N_TILES = N_CORES * TILES_PER_CORE       # 392
NPAD_CORE = TILES_PER_CORE * P           # 6272
PAD_SLOT = 200.0                         # dst_rel sentinel: matches no iota lane
TBL_ROWS = 32768                         # int16 gather-table row limit
GCH = 8                                  # chunks per dma_gather (<=1024 descs/inst)

LAST_RESULTS = []   # test harness reads profiling results from here


def _partition_nodes(deg):
    """LPT-pack nodes into N_TILES bins of <=128 nodes, minimizing max bin
    degree-sum. Returns (tile_of, slot_of, T) with T = uniform chunks/tile."""
    order = np.argsort(-deg, kind="stable")
    heap = [(0, t) for t in range(N_TILES)]
    heapq.heapify(heap)
    counts = np.zeros(N_TILES, np.int64)
    sums = np.zeros(N_TILES, np.int64)
    tile_of = np.empty(N_NODES, np.int64)
    slot_of = np.empty(N_NODES, np.int64)
    for node in order:
        while True:
            s, t = heapq.heappop(heap)
            if counts[t] < P:
                break
        tile_of[node] = t
        slot_of[node] = counts[t]
        counts[t] += 1
        sums[t] += deg[node]
        if counts[t] < P:
            heapq.heappush(heap, (sums[t], t))
    T = int(np.ceil(sums.max() / P))
    return tile_of, slot_of, T


def _build_edge_layout(src, dst, tile_of, slot_of, T):
    """Per-core chunk-major index arrays.

    Returns src_cols, dst_cols: lists (per core) of [P, 49*T] arrays where
    column t*T + j holds chunk j of tile t: lane p is edge j*128+p of that
    tile's padded edge list (src node id / dst slot, PAD entries src=0,
    dst_rel=PAD_SLOT).
    """
    etile = tile_of[dst]
    order = np.argsort(etile, kind="stable")
    counts = np.bincount(etile, minlength=N_TILES)
    src_pad = np.zeros((N_TILES, T * P), np.int64)
    dst_pad = np.full((N_TILES, T * P), PAD_SLOT, np.float32)
    rank = np.arange(N_EDGES) - np.repeat(np.concatenate([[0], np.cumsum(counts)[:-1]]), counts)
    es, ed = src[order], dst[order]
    src_pad[etile[order], rank] = es
    dst_pad[etile[order], rank] = slot_of[ed]
    src_cols, dst_cols = [], []
    for c in range(N_CORES):
        sl = slice(c * TILES_PER_CORE, (c + 1) * TILES_PER_CORE)
        s = src_pad[sl].reshape(TILES_PER_CORE, T, P).transpose(2, 0, 1).reshape(P, TILES_PER_CORE * T)
        d = dst_pad[sl].reshape(TILES_PER_CORE, T, P).transpose(2, 0, 1).reshape(P, TILES_PER_CORE * T)
        src_cols.append(np.ascontiguousarray(s))
        dst_cols.append(np.ascontiguousarray(d))
    return src_cols, dst_cols


def _pick_groups(src_cols, dst_cols, T):
    """Split each core's 49 tiles into contiguous groups whose unique
    source sets fit a 32768-row table. Greedy over tiles, max over cores so
    group bounds are uniform (SPMD). Returns list of (start_tile, end_tile)."""
    bounds = []
    start = 0
    while start < TILES_PER_CORE:
        end = start + 1
        while end < TILES_PER_CORE:
            ok = True
            for c in range(N_CORES):
                blk = src_cols[c][:, start * T:(end + 1) * T]
                pad = dst_cols[c][:, start * T:(end + 1) * T] == PAD_SLOT
                n_uniq = len(np.unique(blk[~pad]))
                if n_uniq > TBL_ROWS - 8:
                    ok = False
                    break
            if not ok:
                break
            end += 1
        bounds.append((start, end))
        start = end
    return bounds


def _build_gather_meta(src_cols, dst_cols, T, bounds):
    """Per (core, group): unique source node list + int16 index array.

    Returns uniqs[c][g] (node ids) and idx16[c] [P, 49*P] int16 where tile
    t's block [:, t*P:(t+1)*P] holds the dma_gather index layout: edge
    i = j*128+p of the tile maps to [i % 16 (replicated x8), t*P + i//16].
    """
    uniqs = [[None] * len(bounds) for _ in range(N_CORES)]
    idx16 = []
    for c in range(N_CORES):
        out = np.zeros((P, TILES_PER_CORE * P), np.int16)
        for g, (t0, t1) in enumerate(bounds):
            blk = src_cols[c][:, t0 * T:t1 * T]
            pad = dst_cols[c][:, t0 * T:t1 * T] == PAD_SLOT
            uniq = np.unique(blk[~pad]) if (~pad).any() else np.array([0], np.int64)
            assert len(uniq) <= TBL_ROWS, f"group {g} core {c}: {len(uniq)} uniques"
            remap = np.zeros(N_NODES, np.int64)
            remap[uniq] = np.arange(len(uniq))
            uniqs[c][g] = uniq
            for t in range(t0, t1):
                arr = src_cols[c][:, t * T:(t + 1) * T]          # [P, T] lane p chunk j
                padm = dst_cols[c][:, t * T:(t + 1) * T] == PAD_SLOT
                r = remap[arr]
                r[padm] = 0
                flat = r.T.reshape(-1)                            # i = j*128+p order
                blk16 = flat.reshape(T * P // 16, 16).T           # [16, T*P/16]
                out[:, t * P:(t + 1) * P] = np.tile(blk16, (8, 1)).astype(np.int16)
        idx16.append(out)
    return uniqs, idx16


def _build_layer1(T, bounds):
    """Layer 1 + p-pretransform as an SPMD bass program."""
    NG = len(bounds)
    nc = bacc.Bacc("TRN2", target_bir_lowering=False, debug=False,
                   enable_asserts=False, num_devices=N_CORES,
                   num_swdge_queues=4, dynamic_dma_scratch_size=65536)
    dt = mybir.dt
    tbls = [nc.dram_tensor(f"tbl{g}", [TBL_ROWS, P], dt.bfloat16, kind="ExternalInput").ap()
            for g in range(NG)]
    idx = nc.dram_tensor("idx", [P, TILES_PER_CORE * P], dt.int16, kind="ExternalInput").ap()
    dst_rel = nc.dram_tensor("dst_rel", [P, TILES_PER_CORE * T], dt.bfloat16, kind="ExternalInput").ap()
    deg_col = nc.dram_tensor("deg_col", [P, TILES_PER_CORE], dt.float32, kind="ExternalInput").ap()
    selfT = nc.dram_tensor("selfT", [P, NPAD_CORE], dt.bfloat16, kind="ExternalInput").ap()
    wl = nc.dram_tensor("wl", [P, DIM_H], dt.bfloat16, kind="ExternalInput").ap()
    wr = nc.dram_tensor("wr", [P, DIM_H], dt.bfloat16, kind="ExternalInput").ap()
    b1 = nc.dram_tensor("b1", [P, 2], dt.float32, kind="ExternalInput").ap()
    w2lT = nc.dram_tensor("w2lT", [P, DIM_OUT * 2], dt.bfloat16, kind="ExternalInput").ap()
    iota = nc.dram_tensor("iota", [P, T * P], dt.bfloat16, kind="ExternalInput").ap()
    identity = nc.dram_tensor("identity", [P, P], dt.bfloat16, kind="ExternalInput").ap()
    hT = nc.dram_tensor("hT", [DIM_H, NPAD_CORE], dt.bfloat16, kind="ExternalOutput").ap()
    pT = nc.dram_tensor("pT", [DIM_OUT, NPAD_CORE], dt.bfloat16, kind="ExternalOutput").ap()

    with tile.TileContext(nc) as tc:
        with ExitStack() as ctx:
            const = ctx.enter_context(tc.tile_pool(name="const", bufs=1))
            msgp = ctx.enter_context(tc.tile_pool(name="msgp", bufs=2))
            sp = ctx.enter_context(tc.tile_pool(name="sp", bufs=2))
            work = ctx.enter_context(tc.tile_pool(name="work", bufs=2))
            outp = ctx.enter_context(tc.tile_pool(name="outp", bufs=3))
            psA = ctx.enter_context(tc.tile_pool(name="psA", bufs=2, space="PSUM"))
            psB = ctx.enter_context(tc.tile_pool(name="psB", bufs=2, space="PSUM"))
            psC = ctx.enter_context(tc.tile_pool(name="psC", bufs=2, space="PSUM"))
            psD = ctx.enter_context(tc.tile_pool(name="psD", bufs=2, space="PSUM"))

            idx_sb = const.tile([P, TILES_PER_CORE * P], dt.int16)
            nc.sync.dma_start(idx_sb[:], idx[:, :])
            dr_sb = const.tile([P, TILES_PER_CORE * T], dt.bfloat16)
            nc.sync.dma_start(dr_sb[:], dst_rel[:, :])
            deg_sb = const.tile([P, TILES_PER_CORE], dt.float32)
            nc.sync.dma_start(deg_sb[:], deg_col[:, :])
            self_sb = const.tile([P, NPAD_CORE], dt.bfloat16)
            nc.sync.dma_start(self_sb[:], selfT[:, :])
            wl_sb = const.tile([P, DIM_H], dt.bfloat16)
            nc.sync.dma_start(wl_sb[:], wl[:, :])
            wr_sb = const.tile([P, DIM_H], dt.bfloat16)
            nc.sync.dma_start(wr_sb[:], wr[:, :])
            b1_sb = const.tile([P, 2], dt.float32)
            nc.sync.dma_start(b1_sb[:], b1[:, :])
            w2l_sb = const.tile([P, DIM_OUT * 2], dt.bfloat16)
            nc.sync.dma_start(w2l_sb[:], w2lT[:, :])
            iota_sb = const.tile([P, T * P], dt.bfloat16)
            nc.sync.dma_start(iota_sb[:], iota[:, :])
            ident = const.tile([P, P], dt.bfloat16)
            nc.sync.dma_start(ident[:], identity[:, :])

            recip = const.tile([P, TILES_PER_CORE], dt.float32)
            nc.vector.tensor_scalar_max(recip[:], deg_sb[:], 1.0)
            nc.vector.reciprocal(recip[:], recip[:])

            for t in range(TILES_PER_CORE):
                g = next(i for i, (t0, t1) in enumerate(bounds) if t0 <= t < t1)
                msgs = msgp.tile([P, T, P], dt.bfloat16)
                for qi, q in enumerate(range(0, T, GCH)):
                    qe = min(q + GCH, T)
                    nc.gpsimd.dma_gather(
                        out_ap=msgs[:, q:qe, :],
                        in_ap=tbls[g][:, :],
                        idxs_ap=idx_sb[:, t * P + q * (P // 16):t * P + qe * (P // 16)],
                        num_idxs=(qe - q) * P,
                        num_idxs_reg=(qe - q) * P,
                        elem_size=P,
                        queue_num=(t * 2 + qi) % 4,
                    )
                S = sp.tile([P, T * P], dt.bfloat16)
                try:
                    nc.vector.tensor_tensor(
                        out=S[:],
                        in0=dr_sb[:, t * T:(t + 1) * T, None].to_broadcast([P, T, P]),
                        in1=iota_sb[:],
                        op=mybir.AluOpType.is_equal,
                    )
                except Exception:
                    for j in range(T):
                        nc.vector.tensor_tensor(
                            out=S[:, j * P:(j + 1) * P],
                            in0=dr_sb[:, t * T + j:t * T + j + 1].to_broadcast([P, P]),
                            in1=iota_sb[:, :P],
                            op=mybir.AluOpType.is_equal,
                        )
                agg_ps = psA.tile([P, P], dt.float32)
                for j in range(T):
                    nc.tensor.matmul(
                        out=agg_ps[:],
                        lhsT=S[:, j * P:(j + 1) * P],
                        rhs=msgs[:, j, :],
                        start=(j == 0),
                        stop=(j == T - 1),
                    )
                agg_sb = work.tile([P, P], dt.bfloat16)
                nc.scalar.mul(agg_sb[:], agg_ps[:], recip[:, t:t + 1])
                aggT_ps = psB.tile([P, P], dt.bfloat16)
                nc.tensor.transpose(out=aggT_ps[:], in_=agg_sb[:], identity=ident[:])
                aggT = work.tile([P, P], dt.bfloat16)
                nc.vector.tensor_copy(aggT[:], aggT_ps[:])
                h_sb = []
                for so in range(2):
                    z_ps = psC.tile([P, P], dt.float32)
                    nc.tensor.matmul(out=z_ps[:], lhsT=wl_sb[:, so * P:(so + 1) * P],
                                     rhs=aggT[:], start=True, stop=False)
                    nc.tensor.matmul(out=z_ps[:], lhsT=wr_sb[:, so * P:(so + 1) * P],
                                     rhs=self_sb[:, t * P:(t + 1) * P], start=False, stop=True)
                    hso = outp.tile([P, P], dt.bfloat16)
                    nc.scalar.activation(hso[:], z_ps[:], mybir.ActivationFunctionType.Relu,
                                         bias=b1_sb[:, so:so + 1], scale=1.0)
                    nc.sync.dma_start(hT[so * P:(so + 1) * P, t * P:(t + 1) * P], hso[:])
                    h_sb.append(hso)
                pT_ps = psD.tile([DIM_OUT, P], dt.float32)
                for si in range(2):
                    nc.tensor.matmul(out=pT_ps[:], lhsT=w2l_sb[:, si * DIM_OUT:(si + 1) * DIM_OUT],
                                     rhs=h_sb[si][:], start=(si == 0), stop=(si == 1))
                pT_sb = outp.tile([DIM_OUT, P], dt.bfloat16)
                nc.vector.tensor_copy(pT_sb[:], pT_ps[:])
                nc.sync.dma_start(pT[:, t * P:(t + 1) * P], pT_sb[:])
    nc.compile()
    return nc


def _build_layer2(T, bounds):
    """Layer 2: mean-aggregate(p) + h @ W2r^T + b2 as an SPMD bass program."""
    NG = len(bounds)
    nc = bacc.Bacc("TRN2", target_bir_lowering=False, debug=False,
                   enable_asserts=False, num_devices=N_CORES,
                   num_swdge_queues=4, dynamic_dma_scratch_size=65536)
    dt = mybir.dt
    tbls = [nc.dram_tensor(f"tbl{g}", [TBL_ROWS, P], dt.bfloat16, kind="ExternalInput").ap()
            for g in range(NG)]
    idx = nc.dram_tensor("idx", [P, TILES_PER_CORE * P], dt.int16, kind="ExternalInput").ap()
    dst_rel = nc.dram_tensor("dst_rel", [P, TILES_PER_CORE * T], dt.bfloat16, kind="ExternalInput").ap()
    deg_col = nc.dram_tensor("deg_col", [P, TILES_PER_CORE], dt.float32, kind="ExternalInput").ap()
    hT0 = nc.dram_tensor("hT0", [P, NPAD_CORE], dt.bfloat16, kind="ExternalInput").ap()
    hT1 = nc.dram_tensor("hT1", [P, NPAD_CORE], dt.bfloat16, kind="ExternalInput").ap()
    w2rT = nc.dram_tensor("w2rT", [P, DIM_OUT * 2], dt.bfloat16, kind="ExternalInput").ap()
    b2row = nc.dram_tensor("b2row", [1, DIM_OUT], dt.bfloat16, kind="ExternalInput").ap()
    ones = nc.dram_tensor("ones", [1, P], dt.bfloat16, kind="ExternalInput").ap()
    iota = nc.dram_tensor("iota", [P, T * P], dt.bfloat16, kind="ExternalInput").ap()
    out = nc.dram_tensor("out", [NPAD_CORE, DIM_OUT], dt.float32, kind="ExternalOutput").ap()

    with tile.TileContext(nc) as tc:
        with ExitStack() as ctx:
            const = ctx.enter_context(tc.tile_pool(name="const", bufs=1))
            msgp = ctx.enter_context(tc.tile_pool(name="msgp", bufs=2))
            sp = ctx.enter_context(tc.tile_pool(name="sp", bufs=2))
            work = ctx.enter_context(tc.tile_pool(name="work", bufs=2))
            outp = ctx.enter_context(tc.tile_pool(name="outp", bufs=3))
            psA = ctx.enter_context(tc.tile_pool(name="psA", bufs=2, space="PSUM"))
            psB = ctx.enter_context(tc.tile_pool(name="psB", bufs=2, space="PSUM"))

            idx_sb = const.tile([P, TILES_PER_CORE * P], dt.int16)
            nc.sync.dma_start(idx_sb[:], idx[:, :])
            dr_sb = const.tile([P, TILES_PER_CORE * T], dt.bfloat16)
            nc.sync.dma_start(dr_sb[:], dst_rel[:, :])
            deg_sb = const.tile([P, TILES_PER_CORE], dt.float32)
            nc.sync.dma_start(deg_sb[:], deg_col[:, :])
            h0_sb = const.tile([P, NPAD_CORE], dt.bfloat16)
            nc.sync.dma_start(h0_sb[:], hT0[:, :])
            h1_sb = const.tile([P, NPAD_CORE], dt.bfloat16)
            nc.sync.dma_start(h1_sb[:], hT1[:, :])
            w2r_sb = const.tile([P, DIM_OUT * 2], dt.bfloat16)
            nc.sync.dma_start(w2r_sb[:], w2rT[:, :])
            b2_sb = const.tile([1, DIM_OUT], dt.bfloat16)
            nc.sync.dma_start(b2_sb[:], b2row[:, :])
            ones_sb = const.tile([1, P], dt.bfloat16)
            nc.sync.dma_start(ones_sb[:], ones[:, :])
            iota_sb = const.tile([P, T * P], dt.bfloat16)
            nc.sync.dma_start(iota_sb[:], iota[:, :])

            recip = const.tile([P, TILES_PER_CORE], dt.float32)
            nc.vector.tensor_scalar_max(recip[:], deg_sb[:], 1.0)
            nc.vector.reciprocal(recip[:], recip[:])

            for t in range(TILES_PER_CORE):
                g = next(i for i, (t0, t1) in enumerate(bounds) if t0 <= t < t1)
                msgs = msgp.tile([P, T, P], dt.bfloat16)
                for qi, q in enumerate(range(0, T, GCH)):
                    qe = min(q + GCH, T)
                    nc.gpsimd.dma_gather(
                        out_ap=msgs[:, q:qe, :],
                        in_ap=tbls[g][:, :],
                        idxs_ap=idx_sb[:, t * P + q * (P // 16):t * P + qe * (P // 16)],
                        num_idxs=(qe - q) * P,
                        num_idxs_reg=(qe - q) * P,
                        elem_size=P,
                        queue_num=(t * 2 + qi) % 4,
                    )
                S = sp.tile([P, T * P], dt.bfloat16)
                try:
                    nc.vector.tensor_tensor(
                        out=S[:],
                        in0=dr_sb[:, t * T:(t + 1) * T, None].to_broadcast([P, T, P]),
                        in1=iota_sb[:],
                        op=mybir.AluOpType.is_equal,
                    )
                except Exception:
                    for j in range(T):
                        nc.vector.tensor_tensor(
                            out=S[:, j * P:(j + 1) * P],
                            in0=dr_sb[:, t * T + j:t * T + j + 1].to_broadcast([P, P]),
                            in1=iota_sb[:, :P],
                            op=mybir.AluOpType.is_equal,
                        )
                agg_ps = psA.tile([P, DIM_OUT], dt.float32)
                for j in range(T):
                    nc.tensor.matmul(
                        out=agg_ps[:],
                        lhsT=S[:, j * P:(j + 1) * P],
                        rhs=msgs[:, j, :DIM_OUT],
                        start=(j == 0),
                        stop=(j == T - 1),
                    )
                agg_sb = work.tile([P, DIM_OUT], dt.float32)
                nc.scalar.mul(agg_sb[:], agg_ps[:], recip[:, t:t + 1])
                z_ps = psB.tile([P, DIM_OUT], dt.float32)
                nc.tensor.matmul(out=z_ps[:], lhsT=h0_sb[:, t * P:(t + 1) * P],
                                 rhs=w2r_sb[:, :DIM_OUT], start=True, stop=False)
                nc.tensor.matmul(out=z_ps[:], lhsT=h1_sb[:, t * P:(t + 1) * P],
                                 rhs=w2r_sb[:, DIM_OUT:], start=False, stop=False)
                nc.tensor.matmul(out=z_ps[:], lhsT=ones_sb[:, :],
                                 rhs=b2_sb[:, :], start=False, stop=True)
                o_sb = outp.tile([P, DIM_OUT], dt.float32)
                nc.vector.tensor_add(o_sb[:], z_ps[:], agg_sb[:])
                nc.sync.dma_start(out[t * P:(t + 1) * P, :], o_sb[:])
    nc.compile()
    return nc


_PROG_CACHE = {}


def _get_programs(T, bounds):
    key = (T, tuple(bounds))
    if key not in _PROG_CACHE:
        l1 = _build_layer1(T, bounds)
        l2 = _build_layer2(T, bounds)
        _PROG_CACHE[key] = (l1, l2)
    return _PROG_CACHE[key]


def kernel(x, edge_index, W1l, W1r, b1, W2l, W2r, b2):
    global LAST_RESULTS
    LAST_RESULTS = []
    x = np.asarray(x, np.float32)
    src = np.asarray(edge_index[0], np.int64)
    dst = np.asarray(edge_index[1], np.int64)

    deg = np.bincount(dst, minlength=N_NODES)
    tile_of, slot_of, T = _partition_nodes(deg)
    src_cols, dst_cols = _build_edge_layout(src, dst, tile_of, slot_of, T)
    bounds = _pick_groups(src_cols, dst_cols, T)
    uniqs, idx16 = _build_gather_meta(src_cols, dst_cols, T, bounds)
    NG = len(bounds)

    pos_of = tile_of * P + slot_of        # global padded slot (core = tile//49)
    l1, l2 = _get_programs(T, bounds)

    trace = bool(int(__import__("os").environ.get("BASS_TRACE", "0") or 0))
    tkw = dict(trace=True, tmpdir=None) if trace else {}

    x_bf = x.astype(BF16)
    iota_np = np.tile(np.arange(P, dtype=np.float32), (P, T)).astype(BF16)

    # per-core metadata
    deg_cols, selfTs, x_tbls = [], [], []
    for c in range(N_CORES):
        sl = slice(c * TILES_PER_CORE, (c + 1) * TILES_PER_CORE)
        dcol = np.zeros((P, TILES_PER_CORE), np.float32)
        sT = np.zeros((NPAD_CORE, DIM_IN), BF16)
        tiles = np.arange(*sl.indices(N_TILES)[:2])
        mask = np.isin(tile_of, tiles)
        nodes = np.nonzero(mask)[0]
        local = (tile_of[nodes] - c * TILES_PER_CORE) * P + slot_of[nodes]
        dcol[slot_of[nodes], tile_of[nodes] - c * TILES_PER_CORE] = deg[nodes]
        sT[local] = x_bf[nodes]
        deg_cols.append(dcol)
        selfTs.append(np.ascontiguousarray(sT.T))
        tbls = []
        for g in range(NG):
            tb = np.zeros((TBL_ROWS, P), BF16)
            tb[:len(uniqs[c][g])] = x_bf[uniqs[c][g]]
            tbls.append(tb)
        x_tbls.append(tbls)

    W1l, W1r, W2l, W2r = (np.asarray(a, np.float32) for a in (W1l, W1r, W2l, W2r))
    wl_p = np.ascontiguousarray(W1l.T).astype(BF16)            # [128, 256]
    wr_p = np.ascontiguousarray(W1r.T).astype(BF16)
    b1_p = np.zeros((P, 2), np.float32)
    b1_p[:, 0] = np.asarray(b1, np.float32)[:P]
    b1_p[:, 1] = np.asarray(b1, np.float32)[P:]
    # w2lT[p, si*64+o] = W2l[o, si*128+p]
    w2l_p = np.concatenate([W2l[:, si * P:(si + 1) * P].T for si in range(2)], axis=1).astype(BF16)
    w2r_p = np.concatenate([W2r[:, si * P:(si + 1) * P].T for si in range(2)], axis=1).astype(BF16)
    b2_p = np.asarray(b2, np.float32).reshape(1, DIM_OUT).astype(BF16)
    ones_p = np.ones((1, P), BF16)

    in_maps = []
    for c in range(N_CORES):
        m = {f"tbl{g}": x_tbls[c][g] for g in range(NG)}
        m.update({
            "idx": idx16[c],
            "dst_rel": dst_cols[c].astype(BF16),
            "deg_col": deg_cols[c],
            "selfT": selfTs[c],
            "wl": wl_p, "wr": wr_p, "b1": b1_p, "w2lT": w2l_p,
            "iota": iota_np, "identity": np.eye(P, dtype=BF16),
        })
        in_maps.append(m)
    r1 = _run_spmd_retry(l1, in_maps, **tkw)
    LAST_RESULTS.append(r1)

    # assemble p gather tables: p rows are indexed by global padded position
    pT_all = np.concatenate([np.asarray(r1.results[c]["pT"]) for c in range(N_CORES)],
                            axis=1)                             # [64, 50176] bf16
    p_rows = np.ascontiguousarray(pT_all.T)                     # [50176, 64]

    in_maps2 = []
    for c in range(N_CORES):
        m = {}
        for g in range(NG):
            tb = np.zeros((TBL_ROWS, P), BF16)
            tb[:len(uniqs[c][g]), :DIM_OUT] = p_rows[pos_of[uniqs[c][g]]]
            m[f"tbl{g}"] = tb
        hT = np.asarray(r1.results[c]["hT"])                    # [256, 6272] bf16
        m.update({
            "idx": idx16[c],
            "dst_rel": dst_cols[c].astype(BF16),
            "deg_col": deg_cols[c],
            "hT0": np.ascontiguousarray(hT[:P]),
            "hT1": np.ascontiguousarray(hT[P:]),
            "w2rT": w2r_p, "b2row": b2_p, "ones": ones_p,
            "iota": iota_np,
        })
        in_maps2.append(m)
    r2 = _run_spmd_retry(l2, in_maps2, **tkw)
    LAST_RESULTS.append(r2)

    big = np.concatenate([np.asarray(r2.results[c]["out"]) for c in range(N_CORES)],
                         axis=0)                                # [50176, 64] f32
    out = np.ascontiguousarray(big[pos_of[np.arange(N_NODES)]], dtype=np.float32)
    return out


# revision 14
# speedup vs baseline: 3.1736x; 1.3880x over previous
"""GraphSAGE (2-layer, mean aggregation) on 8 Trainium2 NeuronCores.

Strategy: destination nodes are sharded across the 8 cores (49 tiles of 128
nodes per core, LPT-balanced by degree). Edges are partitioned by destination
tile, padded to a uniform chunk count T per tile so one SPMD program serves
all cores.

Per-edge source rows are fetched with SWDGE `dma_gather` (two 1024-index
instructions per tile — 1024 descriptors is the per-instruction SWDGE scratch
cap; larger gathers fault the device). dma_gather uses int16 indices (max
32767), so each core's 49 tiles are split into index groups whose unique
source-node sets fit in a 32768-row gather table; tables hold bf16 feature
rows (256 B, the SWDGE minimum element). Measured Q7 descriptor generation
runs ~9 ns/row serially per core, which makes the gathers the span-defining
cost of both layers; the PE/DVE/Scalar work and the DMA transfers themselves
all hide underneath.

The segment sum for a destination tile runs on the PE in bf16: a 0/1
selection matrix S[e, n] = (dst_slot[e] == n) is formed on the vector engine
(iota compare) and S^T @ messages accumulates into PSUM over the tile's
chunks. Mean division, dense lin_l/lin_r matmuls, bias and ReLU happen
on-device in bf16 (fp32 PSUM accumulate).

Layer 2 exploits linearity: p = h @ W2l^T ([N, 64]) is computed at the end of
layer 1 (per-core, own nodes), so layer-2 messages are 64-wide instead of
256-wide — 4x less gather traffic and PE work. Layer 2 then only needs
mean-aggregate(p) + h @ W2r^T + b2, with the bias added via a K=1 matmul
(ones ⊗ b2) into the same PSUM accumulation.

The host does integer index preprocessing, sharding/layout, bf16 casts and
un-sharding; all float tensor math runs on the NeuronCores.
"""
import heapq
import sys
from contextlib import ExitStack

import numpy as np
import ml_dtypes

for _p in ("/opt/trn_rl_repo",):
    if _p not in sys.path:
        sys.path.insert(0, _p)

import concourse.tile as tile
from concourse import bacc, mybir
from concourse.bass_utils import run_bass_kernel_spmd

BF16 = ml_dtypes.bfloat16


def _ensure_axon_hooks():
    """run_bass_kernel_spmd(trace=True) imports antenv.axon_hooks, which this
    image lacks; install a ctypes-backed hook so tracing works (or degrades
    to a no-op instead of an ImportError)."""
    try:
        import antenv.axon_hooks  # noqa: F401
        return
    except ImportError:
        pass
    import contextlib
    import ctypes
    import types

    def _make_hook():
        try:
            lib = ctypes.CDLL("/opt/axon/libaxon_pjrt.so")
        except OSError:
            return None
        if not hasattr(lib, "axon_start_nrt_profile"):
            return None
        lib.axon_start_nrt_profile.argtypes = [ctypes.POINTER(ctypes.c_int64), ctypes.c_size_t]
        lib.axon_start_nrt_profile.restype = ctypes.c_int64
        lib.axon_stop_nrt_profile.argtypes = [ctypes.c_char_p]
        lib.axon_stop_nrt_profile.restype = ctypes.c_int64

        @contextlib.contextmanager
        def _hook(output_dir, device_ids):
            import jax
            jax.devices()
            if device_ids:
                ids = (ctypes.c_int64 * len(device_ids))(*device_ids)
                rc = lib.axon_start_nrt_profile(ids, len(device_ids))
            else:
                rc = lib.axon_start_nrt_profile(None, 0)
            if rc != 0:
                raise RuntimeError(f"axon_start_nrt_profile rc={rc}")
            try:
                yield
            finally:
                lib.axon_stop_nrt_profile(str(output_dir).encode())

        return _hook

    hook = _make_hook()
    mod = types.ModuleType("antenv.axon_hooks")
    mod.get_axon_ntff_profile_hook = lambda: hook
    mod.set_axon_ntff_profile_hook = lambda h: None
    import antenv
    antenv.axon_hooks = mod
    sys.modules["antenv.axon_hooks"] = mod


_ensure_axon_hooks()


def _run_spmd_retry(nc, in_maps, **kw):
    """Retries for transient NRT device errors (axon cores report
    EXEC_UNIT_UNRECOVERABLE for ~60-120 s after a prior faulted run)."""
    import time
    for wait in (75, 120):
        try:
            return run_bass_kernel_spmd(nc, in_maps, core_ids=list(range(N_CORES)), **kw)
        except Exception:
            time.sleep(wait)
    return run_bass_kernel_spmd(nc, in_maps, core_ids=list(range(N_CORES)), **kw)

N_NODES = 50000
N_EDGES = 800000
DIM_IN, DIM_H, DIM_OUT = 128, 256, 64
N_CORES = 8
P = 128
TILES_PER_CORE = 49                      # ceil(50000 / 8 / 128)
N_TILES = N_CORES * TILES_PER_CORE       # 392
NPAD_CORE = TILES_PER_CORE * P           # 6272
PAD_SLOT = 200.0                         # dst_rel sentinel: matches no iota lane
TBL_ROWS = 32768                         # int16 gather-table row limit
GCH = 8                                  # chunks per dma_gather (<=1024 descs/inst)

LAST_RESULTS = []   # test harness reads profiling results from here


def _partition_nodes(deg):
    """LPT-pack nodes into N_TILES bins of <=128 nodes, minimizing max bin
    degree-sum. Returns (tile_of, slot_of, T) with T = uniform chunks/tile."""
    order = np.argsort(-deg, kind="stable")
    heap = [(0, t) for t in range(N_TILES)]
    heapq.heapify(heap)
    counts = np.zeros(N_TILES, np.int64)
    sums = np.zeros(N_TILES, np.int64)
    tile_of = np.empty(N_NODES, np.int64)
    slot_of = np.empty(N_NODES, np.int64)
    for node in order:
        while True:
            s, t = heapq.heappop(heap)
            if counts[t] < P:
                break
        tile_of[node] = t
        slot_of[node] = counts[t]
        counts[t] += 1
        sums[t] += deg[node]
        if counts[t] < P:
            heapq.heappush(heap, (sums[t], t))
    T = int(np.ceil(sums.max() / P))
    return tile_of, slot_of, T


def _build_edge_layout(src, dst, tile_of, slot_of, T):
    """Per-core chunk-major index arrays.

    Returns src_cols, dst_cols: lists (per core) of [P, 49*T] arrays where
    column t*T + j holds chunk j of tile t: lane p is edge j*128+p of that
    tile's padded edge list (src node id / dst slot, PAD entries src=0,
    dst_rel=PAD_SLOT).
    """
    etile = tile_of[dst]
    order = np.argsort(etile, kind="stable")
    counts = np.bincount(etile, minlength=N_TILES)
    src_pad = np.zeros((N_TILES, T * P), np.int64)
    dst_pad = np.full((N_TILES, T * P), PAD_SLOT, np.float32)
    rank = np.arange(N_EDGES) - np.repeat(np.concatenate([[0], np.cumsum(counts)[:-1]]), counts)
    es, ed = src[order], dst[order]
    src_pad[etile[order], rank] = es
    dst_pad[etile[order], rank] = slot_of[ed]
    src_cols, dst_cols = [], []
    for c in range(N_CORES):
        sl = slice(c * TILES_PER_CORE, (c + 1) * TILES_PER_CORE)
        s = src_pad[sl].reshape(TILES_PER_CORE, T, P).transpose(2, 0, 1).reshape(P, TILES_PER_CORE * T)
        d = dst_pad[sl].reshape(TILES_PER_CORE, T, P).transpose(2, 0, 1).reshape(P, TILES_PER_CORE * T)
        src_cols.append(np.ascontiguousarray(s))
        dst_cols.append(np.ascontiguousarray(d))
    return src_cols, dst_cols


def _pick_groups(src_cols, dst_cols, T):
    """Split each core's 49 tiles into contiguous groups whose unique
    source sets fit a 32768-row table. Greedy over tiles, max over cores so
    group bounds are uniform (SPMD). Returns list of (start_tile, end_tile)."""
    bounds = []
    start = 0
    while start < TILES_PER_CORE:
        end = start + 1
        while end < TILES_PER_CORE:
            ok = True
            for c in range(N_CORES):
                blk = src_cols[c][:, start * T:(end + 1) * T]
                pad = dst_cols[c][:, start * T:(end + 1) * T] == PAD_SLOT
                n_uniq = len(np.unique(blk[~pad]))
                if n_uniq > TBL_ROWS - 8:
                    ok = False
                    break
            if not ok:
                break
            end += 1
        bounds.append((start, end))
        start = end
    return bounds


def _build_gather_meta(src_cols, dst_cols, T, bounds):
    """Per (core, group): unique source node list + int16 index array.

    Returns uniqs[c][g] (node ids) and idx16[c] [P, 49*P] int16 where tile
    t's block [:, t*P:(t+1)*P] holds the dma_gather index layout: edge
    i = j*128+p of the tile maps to [i % 16 (replicated x8), t*P + i//16].
    """
    uniqs = [[None] * len(bounds) for _ in range(N_CORES)]
    idx16 = []
    for c in range(N_CORES):
        out = np.zeros((P, TILES_PER_CORE * P), np.int16)
        for g, (t0, t1) in enumerate(bounds):
            blk = src_cols[c][:, t0 * T:t1 * T]
            pad = dst_cols[c][:, t0 * T:t1 * T] == PAD_SLOT
            uniq = np.unique(blk[~pad]) if (~pad).any() else np.array([0], np.int64)
            assert len(uniq) <= TBL_ROWS, f"group {g} core {c}: {len(uniq)} uniques"
            remap = np.zeros(N_NODES, np.int64)
            remap[uniq] = np.arange(len(uniq))
            uniqs[c][g] = uniq
            for t in range(t0, t1):
                arr = src_cols[c][:, t * T:(t + 1) * T]          # [P, T] lane p chunk j
                padm = dst_cols[c][:, t * T:(t + 1) * T] == PAD_SLOT
                r = remap[arr]
                r[padm] = 0
                flat = r.T.reshape(-1)                            # i = j*128+p order
                blk16 = flat.reshape(T * P // 16, 16).T           # [16, T*P/16]
                out[:, t * P:(t + 1) * P] = np.tile(blk16, (8, 1)).astype(np.int16)
        idx16.append(out)
    return uniqs, idx16


def _build_layer1(T, bounds):
    """Layer 1 + p-pretransform as an SPMD bass program."""
    NG = len(bounds)
    nc = bacc.Bacc("TRN2", target_bir_lowering=False, debug=False,
                   enable_asserts=False, num_devices=N_CORES,
                   num_swdge_queues=4, dynamic_dma_scratch_size=65536)
    dt = mybir.dt
    tbls = [nc.dram_tensor(f"tbl{g}", [TBL_ROWS, P], dt.bfloat16, kind="ExternalInput").ap()
            for g in range(NG)]
    idx = nc.dram_tensor("idx", [P, TILES_PER_CORE * P], dt.int16, kind="ExternalInput").ap()
    dst_rel = nc.dram_tensor("dst_rel", [P, TILES_PER_CORE * T], dt.bfloat16, kind="ExternalInput").ap()
    deg_col = nc.dram_tensor("deg_col", [P, TILES_PER_CORE], dt.float32, kind="ExternalInput").ap()
    selfT = nc.dram_tensor("selfT", [P, NPAD_CORE], dt.bfloat16, kind="ExternalInput").ap()
    wl = nc.dram_tensor("wl", [P, DIM_H], dt.bfloat16, kind="ExternalInput").ap()
    wr = nc.dram_tensor("wr", [P, DIM_H], dt.bfloat16, kind="ExternalInput").ap()
    b1 = nc.dram_tensor("b1", [P, 2], dt.float32, kind="ExternalInput").ap()
    w2lT = nc.dram_tensor("w2lT", [P, DIM_OUT * 2], dt.bfloat16, kind="ExternalInput").ap()
    iota = nc.dram_tensor("iota", [P, T * P], dt.bfloat16, kind="ExternalInput").ap()
    identity = nc.dram_tensor("identity", [P, P], dt.bfloat16, kind="ExternalInput").ap()
    hT = nc.dram_tensor("hT", [DIM_H, NPAD_CORE], dt.bfloat16, kind="ExternalOutput").ap()
    pT = nc.dram_tensor("pT", [DIM_OUT, NPAD_CORE], dt.bfloat16, kind="ExternalOutput").ap()

    with tile.TileContext(nc) as tc:
        with ExitStack() as ctx:
            const = ctx.enter_context(tc.tile_pool(name="const", bufs=1))
            msgp = ctx.enter_context(tc.tile_pool(name="msgp", bufs=3))
            sp = ctx.enter_context(tc.tile_pool(name="sp", bufs=3))
            work = ctx.enter_context(tc.tile_pool(name="work", bufs=2))
            outp = ctx.enter_context(tc.tile_pool(name="outp", bufs=3))
            psA = ctx.enter_context(tc.tile_pool(name="psA", bufs=2, space="PSUM"))
            psB = ctx.enter_context(tc.tile_pool(name="psB", bufs=2, space="PSUM"))
            psC = ctx.enter_context(tc.tile_pool(name="psC", bufs=2, space="PSUM"))
            psD = ctx.enter_context(tc.tile_pool(name="psD", bufs=2, space="PSUM"))

            idx_sb = const.tile([P, TILES_PER_CORE * P], dt.int16)
            nc.sync.dma_start(idx_sb[:], idx[:, :])
            dr_sb = const.tile([P, TILES_PER_CORE * T], dt.bfloat16)
            nc.sync.dma_start(dr_sb[:], dst_rel[:, :])
            deg_sb = const.tile([P, TILES_PER_CORE], dt.float32)
            nc.sync.dma_start(deg_sb[:], deg_col[:, :])
            self_sb = const.tile([P, NPAD_CORE], dt.bfloat16)
            nc.sync.dma_start(self_sb[:], selfT[:, :])
            wl_sb = const.tile([P, DIM_H], dt.bfloat16)
            nc.sync.dma_start(wl_sb[:], wl[:, :])
            wr_sb = const.tile([P, DIM_H], dt.bfloat16)
            nc.sync.dma_start(wr_sb[:], wr[:, :])
            b1_sb = const.tile([P, 2], dt.float32)
            nc.sync.dma_start(b1_sb[:], b1[:, :])
            w2l_sb = const.tile([P, DIM_OUT * 2], dt.bfloat16)
            nc.sync.dma_start(w2l_sb[:], w2lT[:, :])
            iota_sb = const.tile([P, T * P], dt.bfloat16)
            nc.sync.dma_start(iota_sb[:], iota[:, :])
            ident = const.tile([P, P], dt.bfloat16)
            nc.sync.dma_start(ident[:], identity[:, :])

            recip = const.tile([P, TILES_PER_CORE], dt.float32)
            nc.vector.tensor_scalar_max(recip[:], deg_sb[:], 1.0)
            nc.vector.reciprocal(recip[:], recip[:])

            for t in range(TILES_PER_CORE):
                g = next(i for i, (t0, t1) in enumerate(bounds) if t0 <= t < t1)
                msgs = msgp.tile([P, T, P], dt.bfloat16)
                for qi, q in enumerate(range(0, T, GCH)):
                    qe = min(q + GCH, T)
                    nc.gpsimd.dma_gather(
                        out_ap=msgs[:, q:qe, :],
                        in_ap=tbls[g][:, :],
                        idxs_ap=idx_sb[:, t * P + q * (P // 16):t * P + qe * (P // 16)],
                        num_idxs=(qe - q) * P,
                        num_idxs_reg=(qe - q) * P,
                        elem_size=P,
                        queue_num=(t * 2 + qi) % 4,
                    )
                S = sp.tile([P, T * P], dt.bfloat16)
                try:
                    nc.vector.tensor_tensor(
                        out=S[:],
                        in0=dr_sb[:, t * T:(t + 1) * T, None].to_broadcast([P, T, P]),
                        in1=iota_sb[:],
                        op=mybir.AluOpType.is_equal,
                    )
                except Exception:
                    for j in range(T):
                        nc.vector.tensor_tensor(
                            out=S[:, j * P:(j + 1) * P],
                            in0=dr_sb[:, t * T + j:t * T + j + 1].to_broadcast([P, P]),
                            in1=iota_sb[:, :P],
                            op=mybir.AluOpType.is_equal,
                        )
                agg_ps = psA.tile([P, P], dt.float32)
                for j in range(T):
                    nc.tensor.matmul(
                        out=agg_ps[:],
                        lhsT=S[:, j * P:(j + 1) * P],
                        rhs=msgs[:, j, :],
                        start=(j == 0),
                        stop=(j == T - 1),
                    )
                agg_sb = work.tile([P, P], dt.bfloat16)
                nc.scalar.mul(agg_sb[:], agg_ps[:], recip[:, t:t + 1])
                aggT_ps = psB.tile([P, P], dt.bfloat16)
                nc.tensor.transpose(out=aggT_ps[:], in_=agg_sb[:], identity=ident[:])
                aggT = work.tile([P, P], dt.bfloat16)
                nc.vector.tensor_copy(aggT[:], aggT_ps[:])
                h_sb = []
                for so in range(2):
                    z_ps = psC.tile([P, P], dt.float32)
                    nc.tensor.matmul(out=z_ps[:], lhsT=wl_sb[:, so * P:(so + 1) * P],
                                     rhs=aggT[:], start=True, stop=False)
                    nc.tensor.matmul(out=z_ps[:], lhsT=wr_sb[:, so * P:(so + 1) * P],
                                     rhs=self_sb[:, t * P:(t + 1) * P], start=False, stop=True)
                    hso = outp.tile([P, P], dt.bfloat16)
                    nc.scalar.activation(hso[:], z_ps[:], mybir.ActivationFunctionType.Relu,
                                         bias=b1_sb[:, so:so + 1], scale=1.0)
                    nc.sync.dma_start(hT[so * P:(so + 1) * P, t * P:(t + 1) * P], hso[:])
                    h_sb.append(hso)
                pT_ps = psD.tile([DIM_OUT, P], dt.float32)
                for si in range(2):
                    nc.tensor.matmul(out=pT_ps[:], lhsT=w2l_sb[:, si * DIM_OUT:(si + 1) * DIM_OUT],
                                     rhs=h_sb[si][:], start=(si == 0), stop=(si == 1))
                pT_sb = outp.tile([DIM_OUT, P], dt.bfloat16)
                nc.vector.tensor_copy(pT_sb[:], pT_ps[:])
                nc.sync.dma_start(pT[:, t * P:(t + 1) * P], pT_sb[:])
    nc.compile()
    return nc


def _build_layer2(T, bounds):
    """Layer 2: mean-aggregate(p) + h @ W2r^T + b2 as an SPMD bass program."""
    NG = len(bounds)
    nc = bacc.Bacc("TRN2", target_bir_lowering=False, debug=False,
                   enable_asserts=False, num_devices=N_CORES,
                   num_swdge_queues=4, dynamic_dma_scratch_size=65536)
    dt = mybir.dt
    tbls = [nc.dram_tensor(f"tbl{g}", [TBL_ROWS, P], dt.bfloat16, kind="ExternalInput").ap()
            for g in range(NG)]
    idx = nc.dram_tensor("idx", [P, TILES_PER_CORE * P], dt.int16, kind="ExternalInput").ap()
    dst_rel = nc.dram_tensor("dst_rel", [P, TILES_PER_CORE * T], dt.bfloat16, kind="ExternalInput").ap()
    deg_col = nc.dram_tensor("deg_col", [P, TILES_PER_CORE], dt.float32, kind="ExternalInput").ap()
    hT0 = nc.dram_tensor("hT0", [P, NPAD_CORE], dt.bfloat16, kind="ExternalInput").ap()
    hT1 = nc.dram_tensor("hT1", [P, NPAD_CORE], dt.bfloat16, kind="ExternalInput").ap()
    w2rT = nc.dram_tensor("w2rT", [P, DIM_OUT * 2], dt.bfloat16, kind="ExternalInput").ap()
    b2row = nc.dram_tensor("b2row", [1, DIM_OUT], dt.bfloat16, kind="ExternalInput").ap()
    ones = nc.dram_tensor("ones", [1, P], dt.bfloat16, kind="ExternalInput").ap()
    iota = nc.dram_tensor("iota", [P, T * P], dt.bfloat16, kind="ExternalInput").ap()
    out = nc.dram_tensor("out", [NPAD_CORE, DIM_OUT], dt.float32, kind="ExternalOutput").ap()

    with tile.TileContext(nc) as tc:
        with ExitStack() as ctx:
            const = ctx.enter_context(tc.tile_pool(name="const", bufs=1))
            msgp = ctx.enter_context(tc.tile_pool(name="msgp", bufs=3))
            sp = ctx.enter_context(tc.tile_pool(name="sp", bufs=3))
            work = ctx.enter_context(tc.tile_pool(name="work", bufs=2))
            outp = ctx.enter_context(tc.tile_pool(name="outp", bufs=3))
            psA = ctx.enter_context(tc.tile_pool(name="psA", bufs=2, space="PSUM"))
            psB = ctx.enter_context(tc.tile_pool(name="psB", bufs=2, space="PSUM"))

            idx_sb = const.tile([P, TILES_PER_CORE * P], dt.int16)
            nc.sync.dma_start(idx_sb[:], idx[:, :])
            dr_sb = const.tile([P, TILES_PER_CORE * T], dt.bfloat16)
            nc.sync.dma_start(dr_sb[:], dst_rel[:, :])
            deg_sb = const.tile([P, TILES_PER_CORE], dt.float32)
            nc.sync.dma_start(deg_sb[:], deg_col[:, :])
            h0_sb = const.tile([P, NPAD_CORE], dt.bfloat16)
            nc.sync.dma_start(h0_sb[:], hT0[:, :])
            h1_sb = const.tile([P, NPAD_CORE], dt.bfloat16)
            nc.sync.dma_start(h1_sb[:], hT1[:, :])
            w2r_sb = const.tile([P, DIM_OUT * 2], dt.bfloat16)
            nc.sync.dma_start(w2r_sb[:], w2rT[:, :])
            b2_sb = const.tile([1, DIM_OUT], dt.bfloat16)
            nc.sync.dma_start(b2_sb[:], b2row[:, :])
            ones_sb = const.tile([1, P], dt.bfloat16)
            nc.sync.dma_start(ones_sb[:], ones[:, :])
            iota_sb = const.tile([P, T * P], dt.bfloat16)
            nc.sync.dma_start(iota_sb[:], iota[:, :])

            recip = const.tile([P, TILES_PER_CORE], dt.float32)
            nc.vector.tensor_scalar_max(recip[:], deg_sb[:], 1.0)
            nc.vector.reciprocal(recip[:], recip[:])

            for t in range(TILES_PER_CORE):
                g = next(i for i, (t0, t1) in enumerate(bounds) if t0 <= t < t1)
                msgs = msgp.tile([P, T, P], dt.bfloat16)
                for qi, q in enumerate(range(0, T, GCH)):
                    qe = min(q + GCH, T)
                    nc.gpsimd.dma_gather(
                        out_ap=msgs[:, q:qe, :],
                        in_ap=tbls[g][:, :],
                        idxs_ap=idx_sb[:, t * P + q * (P // 16):t * P + qe * (P // 16)],
                        num_idxs=(qe - q) * P,
                        num_idxs_reg=(qe - q) * P,
                        elem_size=P,
                        queue_num=(t * 2 + qi) % 4,
                    )
                S = sp.tile([P, T * P], dt.bfloat16)
                try:
                    nc.vector.tensor_tensor(
                        out=S[:],
                        in0=dr_sb[:, t * T:(t + 1) * T, None].to_broadcast([P, T, P]),
                        in1=iota_sb[:],
                        op=mybir.AluOpType.is_equal,
                    )
                except Exception:
                    for j in range(T):
                        nc.vector.tensor_tensor(
                            out=S[:, j * P:(j + 1) * P],
                            in0=dr_sb[:, t * T + j:t * T + j + 1].to_broadcast([P, P]),
                            in1=iota_sb[:, :P],
                            op=mybir.AluOpType.is_equal,
                        )
                agg_ps = psA.tile([P, DIM_OUT], dt.float32)
                for j in range(T):
                    nc.tensor.matmul(
                        out=agg_ps[:],
                        lhsT=S[:, j * P:(j + 1) * P],
                        rhs=msgs[:, j, :DIM_OUT],
                        start=(j == 0),
                        stop=(j == T - 1),
                    )
                agg_sb = work.tile([P, DIM_OUT], dt.float32)
                nc.scalar.mul(agg_sb[:], agg_ps[:], recip[:, t:t + 1])
                z_ps = psB.tile([P, DIM_OUT], dt.float32)
                nc.tensor.matmul(out=z_ps[:], lhsT=h0_sb[:, t * P:(t + 1) * P],
                                 rhs=w2r_sb[:, :DIM_OUT], start=True, stop=False)
                nc.tensor.matmul(out=z_ps[:], lhsT=h1_sb[:, t * P:(t + 1) * P],
                                 rhs=w2r_sb[:, DIM_OUT:], start=False, stop=False)
                nc.tensor.matmul(out=z_ps[:], lhsT=ones_sb[:, :],
                                 rhs=b2_sb[:, :], start=False, stop=True)
                o_sb = outp.tile([P, DIM_OUT], dt.float32)
                nc.vector.tensor_add(o_sb[:], z_ps[:], agg_sb[:])
                nc.sync.dma_start(out[t * P:(t + 1) * P, :], o_sb[:])
    nc.compile()
    return nc


_PROG_CACHE = {}


def _get_programs(T, bounds):
    key = (T, tuple(bounds))
    if key not in _PROG_CACHE:
        l1 = _build_layer1(T, bounds)
        l2 = _build_layer2(T, bounds)
        _PROG_CACHE[key] = (l1, l2)
    return _PROG_CACHE[key]


def kernel(x, edge_index, W1l, W1r, b1, W2l, W2r, b2):
    global LAST_RESULTS
    LAST_RESULTS = []
    x = np.asarray(x, np.float32)
    src = np.asarray(edge_index[0], np.int64)
    dst = np.asarray(edge_index[1], np.int64)

    deg = np.bincount(dst, minlength=N_NODES)
    tile_of, slot_of, T = _partition_nodes(deg)
    src_cols, dst_cols = _build_edge_layout(src, dst, tile_of, slot_of, T)
    bounds = _pick_groups(src_cols, dst_cols, T)
    uniqs, idx16 = _build_gather_meta(src_cols, dst_cols, T, bounds)
    NG = len(bounds)

    pos_of = tile_of * P + slot_of        # global padded slot (core = tile//49)
    l1, l2 = _get_programs(T, bounds)

    trace = bool(int(__import__("os").environ.get("BASS_TRACE", "0") or 0))
    tkw = dict(trace=True, tmpdir=None) if trace else {}

    x_bf = x.astype(BF16)
    iota_np = np.tile(np.arange(P, dtype=np.float32), (P, T)).astype(BF16)

    # per-core metadata
    deg_cols, selfTs, x_tbls = [], [], []
    for c in range(N_CORES):
        sl = slice(c * TILES_PER_CORE, (c + 1) * TILES_PER_CORE)
        dcol = np.zeros((P, TILES_PER_CORE), np.float32)
        sT = np.zeros((NPAD_CORE, DIM_IN), BF16)
        tiles = np.arange(*sl.indices(N_TILES)[:2])
        mask = np.isin(tile_of, tiles)
        nodes = np.nonzero(mask)[0]
        local = (tile_of[nodes] - c * TILES_PER_CORE) * P + slot_of[nodes]
        dcol[slot_of[nodes], tile_of[nodes] - c * TILES_PER_CORE] = deg[nodes]
        sT[local] = x_bf[nodes]
        deg_cols.append(dcol)
        selfTs.append(np.ascontiguousarray(sT.T))
        tbls = []
        for g in range(NG):
            tb = np.zeros((TBL_ROWS, P), BF16)
            tb[:len(uniqs[c][g])] = x_bf[uniqs[c][g]]
            tbls.append(tb)
        x_tbls.append(tbls)

    W1l, W1r, W2l, W2r = (np.asarray(a, np.float32) for a in (W1l, W1r, W2l, W2r))
    wl_p = np.ascontiguousarray(W1l.T).astype(BF16)            # [128, 256]
    wr_p = np.ascontiguousarray(W1r.T).astype(BF16)
    b1_p = np.zeros((P, 2), np.float32)
    b1_p[:, 0] = np.asarray(b1, np.float32)[:P]
    b1_p[:, 1] = np.asarray(b1, np.float32)[P:]
    # w2lT[p, si*64+o] = W2l[o, si*128+p]
    w2l_p = np.concatenate([W2l[:, si * P:(si + 1) * P].T for si in range(2)], axis=1).astype(BF16)
    w2r_p = np.concatenate([W2r[:, si * P:(si + 1) * P].T for si in range(2)], axis=1).astype(BF16)
    b2_p = np.asarray(b2, np.float32).reshape(1, DIM_OUT).astype(BF16)
    ones_p = np.ones((1, P), BF16)

    in_maps = []
    for c in range(N_CORES):
        m = {f"tbl{g}": x_tbls[c][g] for g in range(NG)}
        m.update({
            "idx": idx16[c],
            "dst_rel": dst_cols[c].astype(BF16),
            "deg_col": deg_cols[c],
            "selfT": selfTs[c],
            "wl": wl_p, "wr": wr_p, "b1": b1_p, "w2lT": w2l_p,
            "iota": iota_np, "identity": np.eye(P, dtype=BF16),
        })
        in_maps.append(m)
    r1 = _run_spmd_retry(l1, in_maps, **tkw)
    LAST_RESULTS.append(r1)

    # assemble p gather tables: p rows are indexed by global padded position
    pT_all = np.concatenate([np.asarray(r1.results[c]["pT"]) for c in range(N_CORES)],
                            axis=1)                             # [64, 50176] bf16
    p_rows = np.ascontiguousarray(pT_all.T)                     # [50176, 64]

    in_maps2 = []
    for c in range(N_CORES):
        m = {}
        for g in range(NG):
            tb = np.zeros((TBL_ROWS, P), BF16)
            tb[:len(uniqs[c][g]), :DIM_OUT] = p_rows[pos_of[uniqs[c][g]]]
            m[f"tbl{g}"] = tb
        hT = np.asarray(r1.results[c]["hT"])                    # [256, 6272] bf16
        m.update({
            "idx": idx16[c],
            "dst_rel": dst_cols[c].astype(BF16),
            "deg_col": deg_cols[c],
            "hT0": np.ascontiguousarray(hT[:P]),
            "hT1": np.ascontiguousarray(hT[P:]),
            "w2rT": w2r_p, "b2row": b2_p, "ones": ones_p,
            "iota": iota_np,
        })
        in_maps2.append(m)
    r2 = _run_spmd_retry(l2, in_maps2, **tkw)
    LAST_RESULTS.append(r2)

    big = np.concatenate([np.asarray(r2.results[c]["out"]) for c in range(N_CORES)],
                         axis=0)                                # [50176, 64] f32
    out = np.ascontiguousarray(big[pos_of[np.arange(N_NODES)]], dtype=np.float32)
    return out
